# revision 34
# baseline (speedup 1.0000x reference)
"""Trainium2 Bass kernel for nn_ClassQueryHead (transformer decoder head over
ragged graph batches).

Strategy: data-parallel over graphs (8 graphs per core x 8 cores), with
size-balanced slot packing: graphs sorted by node count, slot j on every core
holds one of ranks [8j, 8j+8), padded to L_j = ceil16(max count in slot) --
~4080 key columns per core instead of 8*640.

Device pipeline per core:
  stage A (shared): self-attn block on class queries, fp8-DoubleRow
    projections (weights+activations e4m3, fp32 accumulate)
  phase 1: K projection for the whole core's packed columns (fp8-DR),
    then per slot: V projection (fp8-DR), scores (fp8 matmul vs shared
    query tile), exp fused on ACT with the ragged mask as per-partition
    bias, denominator via fp8-DR ones-matmul, PV (fp8 matmul)
  phase 2: O-projection (fp8-DR) into the transposed residual stream,
    LayerNorm via ones-matmul partition reductions, FFN in bf16 (fp8
    breaks the 2e-2 error budget), final LN+Linear folded into one
    matmul with wg = out_w * ln_g.
"""
import numpy as np
import ml_dtypes

H = 1024
NH = 16
DH = 64
C = 64
B = 64
FF = 4096
EPS = 1e-5
SCALE = 0.125
NCORES = 8
NG = B // NCORES  # graphs (slots) per core
KC = H // 128     # contract chunks of H
KP = KC // 2      # fp8 DoubleRow pairs
MT = H // 128     # m tiles of H
FM = FF // 128    # ff tiles
NEG = -1e30
F8 = ml_dtypes.float8_e4m3
BF16 = ml_dtypes.bfloat16


def _pieces(n):
    """Split free dim n into pieces <=512."""
    out = []
    rem = n
    while rem > 512:
        out.append(512)
        rem -= 512
    if rem:
        out.append(rem)
    return out


def build_nc(meta, S1, S2):
    import concourse.bass as bass
    import concourse.tile as tile
    import concourse.mybir as mybir
    from concourse import bacc
    from concourse.masks import make_identity

    offs, nchs, TOT_ALLOC = meta
    NCHSUM = sum(nchs)
    f32 = mybir.dt.float32
    f32r = mybir.dt.float32r
    bf16 = mybir.dt.bfloat16
    f8 = mybir.dt.float8e4
    Act = mybir.ActivationFunctionType
    DR = mybir.MatmulPerfMode.DoubleRow

    nc = bacc.Bacc("TRN2", target_bir_lowering=False, debug=False,
                   num_devices=NCORES)

    # ---- DRAM I/O ----
    xt_d = nc.dram_tensor("xt", [H, TOT_ALLOC], f8, kind="ExternalInput")
    mb_d = nc.dram_tensor("mb", [NCHSUM * 128], f32, kind="ExternalInput")
    cq_d = nc.dram_tensor("cq", [C, H], f32, kind="ExternalInput")
    wdr = {}
    for nm in ("sa_wq", "sa_wk", "sa_wv", "sa_wo", "ca_wq", "ca_wk", "ca_wv",
               "ca_wo"):
        wdr[nm] = nc.dram_tensor(nm, [H, H], f8, kind="ExternalInput")
    bdr = {}
    for nm in ("sa_bq", "sa_bk", "sa_bv", "sa_bo", "ca_bq", "ca_bk", "ca_bv",
               "ca_bo", "ln1_g", "ln1_b", "ln2_g", "ln2_b", "ln3_g", "ln3_b",
               "ff_b2"):
        bdr[nm] = nc.dram_tensor(nm, [H], f32, kind="ExternalInput")
    bias10_d = nc.dram_tensor("bias10", [10, H], bf16, kind="ExternalInput")
    pbias_d = nc.dram_tensor("pbias", [128, 80], f32, kind="ExternalInput")
    w1_d = nc.dram_tensor("ff_w1", [H, FF], bf16, kind="ExternalInput")
    b1_d = nc.dram_tensor("ff_b1", [FF], f32, kind="ExternalInput")
    w2_d = nc.dram_tensor("ff_w2", [FF, H], bf16, kind="ExternalInput")
    wg_d = nc.dram_tensor("wg", [H], f32r, kind="ExternalInput")
    sel2_d = nc.dram_tensor("sel2", [2, 128], f32r, kind="ExternalInput")
    out_d = nc.dram_tensor("out", [NG * C], f32, kind="ExternalOutput")

    def bcast_load(nc, out_ap, dram, nparts, offset=0, inner=H):
        src = bass.AP(tensor=dram.ap().tensor, offset=offset,
                      ap=[[0, nparts], [1, inner]])
        nc.gpsimd.dma_start(out=out_ap, in_=src)

    with tile.TileContext(nc) as tc:
        with (
            tc.tile_pool(name="const", bufs=1) as cp,
            tc.tile_pool(name="ps_mm", bufs=2, space="PSUM") as ps_mm,
            tc.tile_pool(name="dram", bufs=2, space="DRAM") as drp,
        ):
            ident = cp.tile([128, 128], f32)
            make_identity(nc, ident[:])
            ones_f = cp.tile([128, 1], f32)
            nc.vector.memset(ones_f[:], 1.0)
            ones_r = cp.tile([128, 1], f32r)
            nc.vector.tensor_copy(ones_r[:], ones_f[:])
            # dual-fp8 LDWEIGHTS needs >=16 cols per k-plane (walrus
            # s3_lw_dual_fp8_restrictions), so the DR ones is 16 wide and
            # only partition 0 of its output is read.
            ones8p = cp.tile([128, 2, 16], f8)
            nc.vector.memset(ones8p[:], 1.0)
            ones8w = cp.tile([128, 16], f8)
            nc.vector.memset(ones8w[:], 1.0)
            ones_b = cp.tile([128, 16], bf16)
            nc.vector.memset(ones_b[:], 1.0)
            sel_eo = []
            for eo in range(2):
                t = cp.tile([1, 128], f32r, tag=f"sel{eo}")
                nc.scalar.dma_start(t[:], sel2_d.ap()[eo, None, :])
                sel_eo.append(t)
            onesrow_f = cp.tile([1, 128], f32)
            nc.vector.memset(onesrow_f[:], 1.0)
            onesrow_r = cp.tile([1, 128], f32r)
            nc.vector.tensor_copy(onesrow_r[:], onesrow_f[:])
            eps_t = cp.tile([128, 1], f32)
            nc.vector.memset(eps_t[:], EPS)

            # per-partition bias tiles, host-packed contiguous [128, 80]
            pb = cp.tile([128, 80], f32)
            nc.scalar.dma_start(pb[:], pbias_d.ap())
            bk_t = pb[:, 0:8]
            bo_t = pb[:, 8:16]
            g3_t = pb[:, 16:24]
            b3_t = pb[:, 24:32]
            b2_t = pb[:, 32:40]
            b1_t = pb[:, 48:80]
            wg_t = cp.tile([128, MT], f32r)
            nc.vector.tensor_copy(wg_t[:], pb[:, 40:48])
            bv_b = cp.tile([128, H], bf16)
            nc.scalar.dma_start(out=bv_b[:], in_=bass.AP(
                tensor=bias10_d.ap().tensor, offset=9 * H,
                ap=[[0, 128], [1, H]]))

            # persistent activations
            x1t = cp.tile([128, KC, C], f32)        # x1 transposed
            qt_eo = cp.tile([128, KC, 2 * C], f8)   # [q_even | q_odd], zero-pad
            nc.vector.memset(qt_eo[:], 0.0)
            x2t = cp.tile([128, MT, NG * C], f32r)  # residual stream T
            sum2_sb = cp.tile([1, NG * C], f32)
            sq2_sb = cp.tile([1, NG * C], f32)

            def ln_row(pool, x, n_p, g_b, b_b, name):
                """LayerNorm on row-layout x [n_p, H] -> new tile."""
                stats = pool.tile([n_p, 2, 6], f32, tag="ln_st")
                for i in range(2):
                    nc.vector.bn_stats(stats[:, i, :], x[:, i * 512:(i + 1) * 512])
                mv = pool.tile([n_p, 2], f32, tag="ln_mv")
                nc.vector.bn_aggr(mv[:], stats[:])
                rstd = pool.tile([n_p, 1], f32, tag="ln_rs")
                nc.scalar.activation(rstd[:], mv[:, 1:2], Act.Sqrt,
                                     bias=eps_t[:n_p])
                nc.vector.reciprocal(rstd[:], rstd[:])
                y = pool.tile([n_p, H], f32, tag="ln_y")
                nc.vector.tensor_scalar(y[:], x[:], scalar1=mv[:, 0:1],
                                        scalar2=rstd[:],
                                        op0=mybir.AluOpType.subtract,
                                        op1=mybir.AluOpType.mult)
                nc.vector.tensor_mul(y[:], y[:], g_b[:])
                nc.vector.tensor_add(y[:], y[:], b_b[:])
                return y

            def transpose_chunks(pool, src, dst_list, n_p=C):
                """PE-transpose src [n_p, H] into dst slices [128, k, n_p]."""
                for k in range(KC):
                    tp = ps_mm.tile([128, 512], f32, tag="acc")
                    nc.tensor.transpose(tp[:, :n_p], src[:, k * 128:(k + 1) * 128],
                                        ident[:n_p, :n_p])
                    for dst, par in dst_list:
                        if par is None:
                            nc.scalar.copy(dst[:, k, :], tp[:, :n_p])
                        elif par == 0:
                            nc.scalar.copy(dst[0:64, k, 0:n_p], tp[0:64, :n_p])
                        else:
                            nc.scalar.copy(dst[64:128, k, n_p:2 * n_p],
                                           tp[64:128, :n_p])

            def load_w8(pool, w_dram, tag="w8", engs=None):
                """Load [H, H] fp8 weight as [128, KC, H]."""
                w_re = w_dram.ap().rearrange("(k p) n -> p k n", p=128)
                w_t = pool.tile([128, KC, H], f8, tag=tag)
                engs = engs or (nc.sync, nc.gpsimd)
                for i, k2 in enumerate(range(0, KC, 2)):
                    eng = engs[i % len(engs)]
                    eng.dma_start(w_t[:, k2:k2 + 2, :], w_re[:, k2:k2 + 2, :])
                return w_t

            def proj_dr(pool, yt8, w8, bias_b, name, out_dt=f32):
                """fp8-DR projection: out [C, H] = y @ W + b (row layout)."""
                o = pool.tile([C, H], out_dt, tag=f"{name}_o")
                for n in range(2):
                    acc = ps_mm.tile([128, 512], f32, tag="acc")
                    for kp in range(KP):
                        nc.tensor.matmul(acc[:C, :], yt8[:, 2 * kp:2 * kp + 2, :],
                                         w8[:, 2 * kp:2 * kp + 2,
                                            n * 512:(n + 1) * 512],
                                         start=(kp == 0), stop=(kp == KP - 1),
                                         perf_mode=DR)
                    nc.vector.tensor_add(o[:, n * 512:(n + 1) * 512],
                                         acc[:C, :], bias_b[:, n * 512:(n + 1) * 512])
                return o

            # ============ STAGE A + PHASE 1 ============
            with tc.tile_pool(name="mid", bufs=1) as midp:
              ots = midp.tile([128, KC, NG, C], f8)  # attn out T, all slots
              with (
                tc.tile_pool(name="ps_st", bufs=2, space="PSUM") as ps_st,
                tc.tile_pool(name="ps_ot", bufs=1, space="PSUM") as ps_ot,
                tc.tile_pool(name="ps_den", bufs=1, space="PSUM") as ps_den,
                tc.tile_pool(name="ps_rdb", bufs=1, space="PSUM") as ps_rdb,
                tc.tile_pool(name="p1", bufs=1) as p1,
                tc.tile_pool(name="vp", bufs=2) as vp,
                tc.tile_pool(name="ptp", bufs=2) as ptp,
              ):
               # phase-1 inputs stream first so the K-projection can start
               # as soon as the tensor queue drains
               mb = p1.tile([128, NCHSUM], f32, tag="mb")
               nc.scalar.dma_start(mb[:], mb_d.ap().rearrange("(p c) -> p c", p=128))
               wkp = tc.alloc_tile_pool(name="wkp", bufs=1)
               wk_t = load_w8(wkp, wdr["ca_wk"], tag="wkc", engs=(nc.sync,))
               xt = p1.tile([128, KC, TOT_ALLOC], f8, tag="xt")
               xt_re = xt_d.ap().rearrange("(k p) n -> p k n", p=128)
               # column-blocked load so the K-projection can start after the
               # first ~1MB block instead of the whole 4MB tensor
               blocks = []
               boff = 0
               while boff < TOT_ALLOC:
                   bw = min(1024, TOT_ALLOC - boff)
                   blocks.append((boff, bw))
                   boff += bw
               for boff, bw in blocks:
                   for k2 in range(0, KC, 2):
                       eng = nc.sync if k2 % 4 == 0 else nc.gpsimd
                       eng.dma_start(xt[:, k2:k2 + 2, boff:boff + bw],
                                     xt_re[:, k2:k2 + 2, boff:boff + bw])
               wv_t = load_w8(p1, wdr["ca_wv"], tag="wvc", engs=(nc.gpsimd,))

               # K-projection for the whole packed column space (dense DR
               # stream, no deps on stage A); wk frees before stage A opens
               kt = p1.tile([128, MT, TOT_ALLOC], f8, tag="kt")
               if True:
                   for boff, bw in blocks:
                       for m in range(MT):
                           off = boff
                           for pc in _pieces(bw):
                               acc = ps_mm.tile([128, 512], f32, tag="acc")
                               for kp in range(KP):
                                   nc.tensor.matmul(
                                       acc[:, :pc],
                                       wk_t[:, 2 * kp:2 * kp + 2,
                                            m * 128:(m + 1) * 128],
                                       xt[:, 2 * kp:2 * kp + 2, off:off + pc],
                                       start=(kp == 0), stop=(kp == KP - 1),
                                       perf_mode=DR)
                               nc.scalar.activation(
                                   kt[:, m, off:off + pc], acc[:, :pc],
                                   Act.Identity, bias=bk_t[:, m:m + 1])
                               off += pc

               wkp.release()
               with (tc.tile_pool(name="sa", bufs=1) as sp,
                     tc.tile_pool(name="wsa", bufs=2) as wsa):
                wq8 = load_w8(wsa, wdr["sa_wq"])
                wk8 = load_w8(wsa, wdr["sa_wk"])
                wv8 = load_w8(wsa, wdr["sa_wv"])
                bb9 = sp.tile([C, 9, H], bf16, tag="bb9")
                nc.scalar.dma_start(out=bb9[:], in_=bass.AP(
                    tensor=bias10_d.ap().tensor, offset=0,
                    ap=[[0, C], [1, 9 * H]]))
                bias_bcast = {}
                for bi, nm in enumerate(("sa_bq", "sa_bk", "sa_bv", "sa_bo",
                                         "ca_bq", "ln1_g", "ln1_b", "ln2_g",
                                         "ln2_b")):
                    bias_bcast[nm] = bb9[:, bi, :]

                x0 = sp.tile([C, H], f32)
                nc.scalar.dma_start(x0[:], cq_d.ap())
                y1 = ln_row(sp, x0, C, bias_bcast["ln1_g"], bias_bcast["ln1_b"], "ln1")
                y1t = sp.tile([128, KC, C], f8)
                transpose_chunks(sp, y1, [(y1t, None)])
                q1 = proj_dr(sp, y1t, wq8, bias_bcast["sa_bq"], "q1")
                k1 = proj_dr(sp, y1t, wk8, bias_bcast["sa_bk"], "k1")
                v1 = proj_dr(sp, y1t, wv8, bias_bcast["sa_bv"], "v1")

                k1t = sp.tile([128, KC, C], bf16)
                transpose_chunks(sp, k1, [(k1t, None)])
                q1t_eo = sp.tile([128, KC, 2 * C], bf16)
                nc.vector.memset(q1t_eo[:], 0.0)
                transpose_chunks(sp, q1, [(q1t_eo, 0), (q1t_eo, 1)])
                v1b = sp.tile([128, NH, DH], bf16)
                nc.vector.memset(v1b[:], 0.0)
                nc.vector.tensor_copy(
                    v1b[0:64, :, :], v1[:].rearrange("p (h d) -> p h d", d=DH))

                # self-attn scores/exp (keys=64, one chunk)
                pt1 = sp.tile([128, NH, C], bf16)
                nc.vector.memset(pt1[:], 0.0)
                for half in range(2):
                    st = ps_st.tile([128, 4, 2 * C], f32, tag="st")
                    for i in range(4):
                        t = half * 4 + i
                        nc.tensor.matmul(st[:C, i, :], k1t[:, t, :],
                                         q1t_eo[:, t, :], start=True, stop=True)
                    nc.scalar.activation(
                        pt1[0:C, half * 8:(half + 1) * 8, :],
                        st[:C, :, :].rearrange("p a b -> p (a b)").rearrange(
                            "p (h c) -> p h c", c=C),
                        Act.Exp, bias=0.0, scale=SCALE)
                dsb = [sp.tile([1, 512], f32r, tag=f"dsb{e}", name=f"dsb_a{e}")
                       for e in range(2)]
                for hf in range(2):
                    den1 = ps_den.tile([16, 512], f32, tag="den")
                    nc.tensor.matmul(
                        den1[:], ones_b[:],
                        pt1[:, hf * 8:(hf + 1) * 8, :].rearrange(
                            "p h c -> p (h c)"),
                        start=True, stop=True)
                    with nc.allow_low_precision(reason="f32r rden for bcast matmul"):
                        nc.vector.reciprocal(dsb[hf][:], den1[0:1, :])
                ot1 = ps_ot.tile([128, KC, 2 * C], f32, tag="ot")
                for t in range(KC):
                    nc.tensor.matmul(
                        ot1[:, t, :],
                        v1b[:, 2 * t:2 * t + 2, :].rearrange("p a d -> p (a d)"),
                        pt1[:, 2 * t:2 * t + 2, :].rearrange("p a c -> p (a c)"),
                        start=True, stop=True)
                # broadcast 1/den across partitions: accumulate
                # sel_e⊗even + sel_o⊗odd heads (full-128 dst, ISA-safe);
                # dsb half hf holds heads 8hf..8hf+7 -> strided eo slice
                rdb1 = ps_rdb.tile([128, KC * C], f32, tag="rdb")
                for hf in range(2):
                    dv = dsb[hf][:].rearrange("p (h c) -> p h c", c=C)
                    for eo in range(2):
                        nc.tensor.matmul(rdb1[:, hf * 256:(hf + 1) * 256],
                                         sel_eo[eo][:], dv[:, eo::2, :],
                                         start=(eo == 0), stop=(eo == 1))
                ot1s = sp.tile([128, KC, C], f8)
                rdb1_sb = sp.tile([128, KC, C], f32, tag="rdb1_sb")
                nc.scalar.copy(rdb1_sb[:].rearrange("p t c -> p (t c)"), rdb1[:])
                nc.vector.tensor_mul(ot1s[0:64], ot1[0:64, :, 0:C], rdb1_sb[0:64])
                nc.vector.tensor_mul(ot1s[64:128], ot1[64:128, :, C:2 * C],
                                     rdb1_sb[64:128])

                # O-proj + residual -> x1 row layout (fp8-DR)
                wo8 = load_w8(wsa, wdr["sa_wo"])
                x1 = sp.tile([C, H], f32)
                for n in range(2):
                    acc = ps_mm.tile([128, 512], f32, tag="acc")
                    for kp in range(KP):
                        nc.tensor.matmul(acc[:C, :], ot1s[:, 2 * kp:2 * kp + 2, :],
                                         wo8[:, 2 * kp:2 * kp + 2,
                                             n * 512:(n + 1) * 512],
                                         start=(kp == 0), stop=(kp == KP - 1),
                                         perf_mode=DR)
                    nc.vector.tensor_add(x1[:, n * 512:(n + 1) * 512], acc[:C, :],
                                         bias_bcast["sa_bo"][:, n * 512:(n + 1) * 512])
                    nc.vector.tensor_add(x1[:, n * 512:(n + 1) * 512],
                                         x1[:, n * 512:(n + 1) * 512],
                                         x0[:, n * 512:(n + 1) * 512])

                y2 = ln_row(sp, x1, C, bias_bcast["ln2_g"], bias_bcast["ln2_b"], "ln2")
                y2t = sp.tile([128, KC, C], f8)
                transpose_chunks(sp, y2, [(y2t, None)])
                wq8c = load_w8(wsa, wdr["ca_wq"])
                qca = proj_dr(sp, y2t, wq8c, bias_bcast["ca_bq"], "q1")
                transpose_chunks(sp, qca, [(qt_eo, 0), (qt_eo, 1)])
                transpose_chunks(sp, x1, [(x1t, None)])

               # ============ PHASE 1: per-slot cross-attention ============
               if True:
                    def compute_v(g):
                        goff, nch = offs[g], nchs[g]
                        v = vp.tile([128, nchs[0], NH, DH], f8, tag="v",
                                    name=f"v_{g}")
                        for ch in range(nch):
                            coff = goff + ch * 128
                            for half in range(2):
                                acc = ps_mm.tile([128, 512], f32, tag="acc")
                                for kp in range(KP):
                                    nc.tensor.matmul(
                                        acc[:],
                                        xt[:, 2 * kp:2 * kp + 2, coff:coff + 128],
                                        wv_t[:, 2 * kp:2 * kp + 2,
                                             half * 512:(half + 1) * 512],
                                        start=(kp == 0), stop=(kp == KP - 1),
                                        perf_mode=DR)
                                nc.vector.tensor_add(
                                    v[:, ch, half * 8:(half + 1) * 8, :],
                                    acc[:].rearrange("p (h d) -> p h d", d=DH),
                                    bv_b[:, half * 512:(half + 1) * 512].rearrange(
                                        "p (h d) -> p h d", d=DH))
                        return v

                    cc0 = 0  # running chunk index into mb
                    v_next = compute_v(0)
                    for g in range(NG):
                        goff, nch = offs[g], nchs[g]
                        v = v_next

                        pt = ptp.tile([128, nchs[0], NH, C], f8, tag="pt")
                        for ch in range(nch):
                            coff = goff + ch * 128
                            for half in range(2):
                                st = ps_st.tile([128, 4, 2 * C], f32, tag="st")
                                for i in range(4):
                                    t = half * 4 + i
                                    nc.tensor.matmul(
                                        st[:, i, :],
                                        kt[:, t, coff:coff + 128],
                                        qt_eo[:, t, :],
                                        start=True, stop=True)
                                nc.scalar.activation(
                                    pt[:, ch, half * 8:(half + 1) * 8, :],
                                    st[:].rearrange("p a b -> p (a b)").rearrange(
                                        "p (h c) -> p h c", c=C),
                                    Act.Exp, bias=mb[:, cc0 + ch:cc0 + ch + 1],
                                    scale=SCALE)

                        if g + 1 < NG:
                            v_next = compute_v(g + 1)

                        dsb2 = [p1.tile([1, 512], f32r, tag=f"dsb{e}",
                                        name=f"dsb2_{e}") for e in range(2)]
                        nDR = nch // 2
                        for hf in range(2):
                            hs = slice(hf * 8, (hf + 1) * 8)
                            den = ps_den.tile([16, 512], f32, tag="den")
                            for cp2 in range(nDR):
                                nc.tensor.matmul(
                                    den[:], ones8p[:],
                                    pt[:, 2 * cp2:2 * cp2 + 2, hs, :].rearrange(
                                        "p a h c -> p a (h c)"),
                                    start=(cp2 == 0), stop=(nch % 2 == 0 and
                                                            cp2 == nDR - 1),
                                    perf_mode=DR)
                            if nch % 2:
                                nc.tensor.matmul(
                                    den[:], ones8w[:],
                                    pt[:, nch - 1, hs, :].rearrange(
                                        "p h c -> p (h c)"),
                                    start=(nDR == 0), stop=True)
                            with nc.allow_low_precision(
                                    reason="f32r rden for bcast matmul"):
                                nc.vector.reciprocal(dsb2[hf][:], den[0:1, :])
                        ot = ps_ot.tile([128, KC, 2 * C], f32, tag="ot")
                        for t in range(KC):
                            for cp2 in range(nDR):
                                nc.tensor.matmul(
                                    ot[:, t, :],
                                    v[:, 2 * cp2:2 * cp2 + 2, 2 * t:2 * t + 2,
                                      :].rearrange("p a b d -> p a (b d)"),
                                    pt[:, 2 * cp2:2 * cp2 + 2, 2 * t:2 * t + 2,
                                       :].rearrange("p a b c -> p a (b c)"),
                                    start=(cp2 == 0), stop=(nch % 2 == 0 and
                                                            cp2 == nDR - 1),
                                    perf_mode=DR)
                            if nch % 2:
                                nc.tensor.matmul(
                                    ot[:, t, :],
                                    v[:, nch - 1, 2 * t:2 * t + 2, :].rearrange(
                                        "p a d -> p (a d)"),
                                    pt[:, nch - 1, 2 * t:2 * t + 2, :].rearrange(
                                        "p a c -> p (a c)"),
                                    start=(nDR == 0), stop=True)
                        rdb = ps_rdb.tile([128, KC * C], f32, tag="rdb")
                        for hf in range(2):
                            dv = dsb2[hf][:].rearrange("p (h c) -> p h c", c=C)
                            for eo in range(2):
                                nc.tensor.matmul(
                                    rdb[:, hf * 256:(hf + 1) * 256],
                                    sel_eo[eo][:], dv[:, eo::2, :],
                                    start=(eo == 0), stop=(eo == 1))
                        rdb_sb = p1.tile([128, KC, C], f32, tag="rdb_sb")
                        nc.scalar.copy(rdb_sb[:].rearrange("p t c -> p (t c)"),
                                       rdb[:])
                        nc.vector.tensor_mul(ots[0:64, :, g, :],
                                             ot[0:64, :, 0:C], rdb_sb[0:64])
                        nc.vector.tensor_mul(ots[64:128, :, g, :],
                                             ot[64:128, :, C:2 * C],
                                             rdb_sb[64:128])
                        cc0 += nch

              # ============ PHASE 2a: O-projection (fp8-DR) ============
              with (tc.tile_pool(name="wop", bufs=1) as wop,
                    tc.tile_pool(name="sq0p", bufs=2) as sq0p,
                    tc.tile_pool(name="ps_st0", bufs=2, space="PSUM") as ps_st0):
                sum_ps0 = ps_st0.tile([1, NG * C], f32, tag="stat0")
                sq_ps0 = ps_st0.tile([1, NG * C], f32, tag="stat0")
                wo_t = load_w8(wop, wdr["ca_wo"], tag="woc")
                for m in range(MT):
                    acc = ps_mm.tile([128, 512], f32, tag="acc")
                    for kp in range(KP):
                        nc.tensor.matmul(
                            acc[:],
                            wo_t[:, 2 * kp:2 * kp + 2, m * 128:(m + 1) * 128],
                            ots[:, 2 * kp:2 * kp + 2, :, :].rearrange(
                                "p a g c -> p a (g c)"),
                            start=(kp == 0), stop=(kp == KP - 1),
                            perf_mode=DR)
                    nc.scalar.activation(x2t[:, m, :], acc[:], Act.Identity,
                                         bias=bo_t[:, m:m + 1])
                    nc.vector.tensor_add(
                        x2t[:, m, :].rearrange("p (g c) -> p g c", c=C),
                        x2t[:, m, :].rearrange("p (g c) -> p g c", c=C),
                        x1t[:, m, None, :].to_broadcast((128, NG, C)))
                    sq0 = sq0p.tile([128, NG * C], f32r, tag="sq0")
                    nc.vector.tensor_mul(sq0[:], x2t[:, m, :], x2t[:, m, :])
                    nc.tensor.matmul(sum_ps0[:], ones_r[:], x2t[:, m, :],
                                     start=(m == 0), stop=(m == MT - 1))
                    nc.tensor.matmul(sq_ps0[:], ones_r[:], sq0[:],
                                     start=(m == 0), stop=(m == MT - 1))
                nc.vector.tensor_copy(sum2_sb[:], sum_ps0[:])
                nc.vector.tensor_copy(sq2_sb[:], sq_ps0[:])

            # ============ PHASE 2: FFN (bf16), output ============
            with (
                tc.tile_pool(name="p2", bufs=1) as p2,
                tc.tile_pool(name="wstr", bufs=3) as wstr,
                tc.tile_pool(name="wstr2", bufs=2) as wstr2,
                tc.tile_pool(name="sq", bufs=2) as sqp,
                tc.tile_pool(name="ps_stat", bufs=4, space="PSUM") as ps_stat,
                tc.tile_pool(name="ps_bc", bufs=2, space="PSUM") as ps_bc,
            ):
                R = NG * C  # 512 rows
                # LN3 stats were accumulated during O-proj (sum2_sb/sq2_sb)
                mean = p2.tile([1, R], f32r, tag="mean")
                with nc.allow_low_precision(reason="f32r mean/rstd for K=1 bcast matmul"):
                    nc.scalar.mul(mean[:], sum2_sb[:], 1.0 / H)
                var = p2.tile([1, R], f32, tag="var")
                nc.scalar.mul(var[:], sq2_sb[:], 1.0 / H)
                m2 = p2.tile([1, R], f32, tag="m2")
                nc.vector.tensor_mul(m2[:], mean[:], mean[:])
                nc.vector.tensor_sub(var[:], var[:], m2[:])
                rstd = p2.tile([1, R], f32r, tag="rstd")
                with nc.allow_low_precision(reason="f32r mean/rstd for K=1 bcast matmul"):
                    nc.scalar.activation(rstd[:], var[:], Act.Sqrt, bias=eps_t[0:1])
                    nc.vector.reciprocal(rstd[:], rstd[:])
                mean_b = ps_bc.tile([128, R], f32, tag="bc")
                rstd_b = ps_bc.tile([128, R], f32, tag="bc")
                nc.tensor.matmul(mean_b[:], onesrow_r[:], mean[:],
                                 start=True, stop=True)
                nc.tensor.matmul(rstd_b[:], onesrow_r[:], rstd[:],
                                 start=True, stop=True)

                y3t = p2.tile([128, KC, R], bf16, tag="y3t")
                for m in range(MT):
                    nc.vector.tensor_sub(y3t[:, m, :], x2t[:, m, :], mean_b[:])
                    nc.vector.tensor_mul(y3t[:, m, :], y3t[:, m, :], rstd_b[:])
                    nc.vector.tensor_scalar(
                        y3t[:, m, :], y3t[:, m, :],
                        scalar1=g3_t[:, m:m + 1], scalar2=b3_t[:, m:m + 1],
                        op0=mybir.AluOpType.mult, op1=mybir.AluOpType.add)

                # GEMM1: h1T [128, FM, R] bf16
                h1t = p2.tile([128, FM, R], bf16, tag="h1t")
                w1_re = w1_d.ap().rearrange("(k p) f -> p k f", p=128)
                for fm in range(FM):
                    w1c = wstr.tile([128, KC, 128], bf16, tag="w1c")
                    for k4 in range(0, KC, 4):
                        nc.sync.dma_start(w1c[:, k4:k4 + 4, :],
                                          w1_re[:, k4:k4 + 4, fm * 128:(fm + 1) * 128])
                    acc = ps_mm.tile([128, 512], f32, tag="acc")
                    for k in range(KC):
                        nc.tensor.matmul(acc[:], w1c[:, k, :], y3t[:, k, :],
                                         start=(k == 0), stop=(k == KC - 1))
                    nc.scalar.activation(h1t[:, fm, :], acc[:], Act.Relu,
                                         bias=b1_t[:, fm:fm + 1])

                # GEMM2: x3T = W2^T-chunks @ h1T + x2T + b2
                sum3 = ps_stat.tile([1, R], f32, tag="stat")
                sq3 = ps_stat.tile([1, R], f32, tag="stat")
                a_ps = ps_stat.tile([1, R], f32, tag="stat")
                x3t = p2.tile([128, MT, R], f32r, tag="x3t")
                w2_re = w2_d.ap().rearrange("(k p) f -> p k f", p=128)
                w2cs = {}

                def load_w2c(m):
                    w2c = wstr2.tile([128, FM, 128], bf16, tag="w2c")
                    for f8_ in range(0, FM, 8):
                        nc.gpsimd.dma_start(
                            w2c[:, f8_:f8_ + 8, :],
                            w2_re[:, f8_:f8_ + 8, m * 128:(m + 1) * 128])
                    w2cs[m] = w2c
                load_w2c(0)
                for m in range(MT):
                    w2c = w2cs.pop(m)
                    if m + 1 < MT:
                        load_w2c(m + 1)
                    acc = ps_mm.tile([128, 512], f32, tag="acc")
                    for fk in range(FM):
                        nc.tensor.matmul(acc[:], w2c[:, fk, :], h1t[:, fk, :],
                                         start=(fk == 0), stop=(fk == FM - 1))
                    nc.scalar.activation(x3t[:, m, :], acc[:], Act.Identity,
                                         bias=b2_t[:, m:m + 1])
                    nc.vector.tensor_add(x3t[:, m, :], x3t[:, m, :],
                                         x2t[:, m, :])
                    sq = sqp.tile([128, R], f32r, tag="sq")
                    nc.vector.tensor_mul(sq[:], x3t[:, m, :], x3t[:, m, :])
                    nc.tensor.matmul(sum3[:], ones_r[:], x3t[:, m, :],
                                     start=(m == 0), stop=(m == MT - 1))
                    nc.tensor.matmul(sq3[:], ones_r[:], sq[:],
                                     start=(m == 0), stop=(m == MT - 1))
                    nc.tensor.matmul(a_ps[:], wg_t[:, m:m + 1], x3t[:, m, :],
                                     start=(m == 0), stop=(m == MT - 1))

                # final LN + linear folded: logits = rstd*(A - mean*S1) + S2
                mean3 = p2.tile([1, R], f32, tag="mean3")
                nc.scalar.mul(mean3[:], sum3[:], 1.0 / H)
                var3 = p2.tile([1, R], f32, tag="var3")
                nc.scalar.mul(var3[:], sq3[:], 1.0 / H)
                m23 = p2.tile([1, R], f32, tag="m23")
                nc.vector.tensor_mul(m23[:], mean3[:], mean3[:])
                nc.vector.tensor_sub(var3[:], var3[:], m23[:])
                rstd3 = p2.tile([1, R], f32, tag="rstd3")
                nc.scalar.activation(rstd3[:], var3[:], Act.Sqrt, bias=eps_t[0:1])
                nc.vector.reciprocal(rstd3[:], rstd3[:])
                logits = p2.tile([1, R], f32, tag="logits")
                nc.scalar.mul(logits[:], mean3[:], -S1)
                nc.vector.tensor_add(logits[:], logits[:], a_ps[:])
                nc.vector.tensor_mul(logits[:], logits[:], rstd3[:])
                nc.scalar.add(logits[:], logits[:], S2)
                nc.sync.dma_start(out_d.ap()[None, :], logits[:])

    nc.compile()
    return nc


def _ceil16(n):
    return -(-int(n) // 16) * 16


def _prep(inputs):
    nf = np.ascontiguousarray(np.asarray(inputs["node_features"], np.float32))
    batch = np.asarray(inputs["batch"]).astype(np.int64)
    counts = np.bincount(batch, minlength=B)
    offsets = np.concatenate([[0], np.cumsum(counts)[:-1]])

    # slot packing: rank graphs by count desc; slot j holds ranks [8j, 8j+8)
    order = np.argsort(-counts, kind="stable")
    L = [_ceil16(max(int(counts[order[j * NCORES]]), 16)) for j in range(NG)]
    offs = np.concatenate([[0], np.cumsum(L)[:-1]]).astype(int).tolist()
    nchs = [-(-l // 128) for l in L]
    TOT_ALLOC = _ceil16(offs[-1] + nchs[-1] * 128)
    meta = (offs, nchs, TOT_ALLOC)

    xts, mbs = [], []
    NCHSUM = sum(nchs)
    for c in range(NCORES):
        xt = np.zeros((H, TOT_ALLOC), np.float32)
        mbm = np.full((NCHSUM, 128), NEG, np.float32)
        cc0 = 0
        for j in range(NG):
            g = int(order[j * NCORES + c])
            n = int(counts[g])
            o = int(offsets[g])
            xt[:, offs[j]:offs[j] + n] = nf[o:o + n].T
            # empty graph guard: unmask one zero-feature key
            nv = max(n, 1)
            for ch in range(nchs[j]):
                lo = ch * 128
                mbm[cc0 + ch, :max(0, min(nv - lo, 128))] = 0.0
            cc0 += nchs[j]
        xts.append(np.clip(xt, -240.0, 240.0).astype(F8))
        mbs.append(np.ascontiguousarray(mbm.T.reshape(-1)))

    out_w = np.asarray(inputs["out_w"], np.float32)[:, 0]
    og = np.asarray(inputs["out_ln_g"], np.float32)
    ob = np.asarray(inputs["out_ln_b"], np.float32)
    wg = (out_w * og).astype(np.float32)
    S1 = float(wg.sum())
    S2 = float((out_w * ob).sum() + np.asarray(inputs["out_b"], np.float32)[0])

    def perm128(v):
        return np.asarray(v, np.float32).reshape(-1, 128).T

    pbias = np.zeros((128, 80), np.float32)
    pbias[:, 0:8] = perm128(inputs["ca_bk"])
    pbias[:, 8:16] = perm128(inputs["ca_bo"])
    pbias[:, 16:24] = perm128(inputs["ln3_g"])
    pbias[:, 24:32] = perm128(inputs["ln3_b"])
    pbias[:, 32:40] = perm128(inputs["ff_b2"])
    pbias[:, 40:48] = perm128(wg)
    pbias[:, 48:80] = perm128(inputs["ff_b1"])
    sel2 = np.zeros((2, 128), np.float32)
    sel2[0, 0:64] = 1.0
    sel2[1, 64:128] = 1.0
    common = {"cq": np.ascontiguousarray(np.asarray(inputs["class_queries"], np.float32)),
              "wg": wg, "sel2": sel2, "pbias": pbias}
    for nm in ("sa_wq", "sa_wk", "sa_wv", "sa_wo", "ca_wq", "ca_wk", "ca_wv",
               "ca_wo"):
        common[nm] = np.clip(np.asarray(inputs[nm], np.float32),
                             -240.0, 240.0).astype(F8)
    for nm in ("ff_w1", "ff_w2"):
        common[nm] = np.asarray(inputs[nm], np.float32).astype(BF16)
    for nm in ("sa_bq", "sa_bk", "sa_bv", "sa_bo", "ca_bq", "ca_bk",
               "ca_bv", "ca_bo", "ln1_g", "ln1_b", "ln2_g", "ln2_b",
               "ln3_g", "ln3_b", "ff_b1", "ff_b2"):
        common[nm] = np.ascontiguousarray(np.asarray(inputs[nm], np.float32))
    common["bias10"] = np.stack(
        [np.asarray(inputs[nm], np.float32) for nm in
         ("sa_bq", "sa_bk", "sa_bv", "sa_bo", "ca_bq",
          "ln1_g", "ln1_b", "ln2_g", "ln2_b", "ca_bv")]).astype(BF16)

    in_maps = []
    for c in range(NCORES):
        m = dict(common)
        m["xt"] = xts[c]
        m["mb"] = mbs[c]
        in_maps.append(m)
    return meta, S1, S2, in_maps, order


def _run(inputs, trace=False):
    from concourse.bass_utils import run_bass_kernel_spmd
    meta, S1, S2, in_maps, order = _prep(inputs)
    nc = build_nc(meta, S1, S2)
    try:
        r = run_bass_kernel_spmd(nc, in_maps, core_ids=list(range(NCORES)),
                                 trace=trace)
    except Exception:
        # transient device wedge (NRT_EXEC_UNIT_UNRECOVERABLE) clears on retry
        r = run_bass_kernel_spmd(nc, in_maps, core_ids=list(range(NCORES)),
                                 trace=trace)
    out = np.zeros((B, C), np.float32)
    for c in range(NCORES):
        rc = r.results[c]["out"].reshape(NG, C)
        for j in range(NG):
            out[int(order[j * NCORES + c])] = rc[j]
    return out.astype(np.float32), r


def kernel(**inputs):
    return _run(inputs, trace=False)[0]


# revision 35
# speedup vs baseline: 1.0230x; 1.0230x over previous
"""Trainium2 Bass kernel for nn_ClassQueryHead (transformer decoder head over
ragged graph batches).

Strategy: data-parallel over graphs (8 graphs per core x 8 cores), with
size-balanced slot packing: graphs sorted by node count, slot j on every core
holds one of ranks [8j, 8j+8), padded to L_j = ceil16(max count in slot) --
~4080 key columns per core instead of 8*640.

Device pipeline per core:
  stage A (shared): self-attn block on class queries, fp8-DoubleRow
    projections (weights+activations e4m3, fp32 accumulate)
  phase 1: K projection for the whole core's packed columns (fp8-DR),
    then per slot: V projection (fp8-DR), scores (fp8 matmul vs shared
    query tile), exp fused on ACT with the ragged mask as per-partition
    bias, denominator via fp8-DR ones-matmul, PV (fp8 matmul)
  phase 2: O-projection (fp8-DR) into the transposed residual stream,
    LayerNorm via ones-matmul partition reductions, FFN in bf16 (fp8
    breaks the 2e-2 error budget), final LN+Linear folded into one
    matmul with wg = out_w * ln_g.
"""
import numpy as np
import ml_dtypes

H = 1024
NH = 16
DH = 64
C = 64
B = 64
FF = 4096
EPS = 1e-5
SCALE = 0.125
NCORES = 8
NG = B // NCORES  # graphs (slots) per core
KC = H // 128     # contract chunks of H
KP = KC // 2      # fp8 DoubleRow pairs
MT = H // 128     # m tiles of H
FM = FF // 128    # ff tiles
NEG = -1e30
F8 = ml_dtypes.float8_e4m3
BF16 = ml_dtypes.bfloat16


def _pieces(n):
    """Split free dim n into pieces <=512."""
    out = []
    rem = n
    while rem > 512:
        out.append(512)
        rem -= 512
    if rem:
        out.append(rem)
    return out


def build_nc(meta, S1, S2):
    import concourse.bass as bass
    import concourse.tile as tile
    import concourse.mybir as mybir
    from concourse import bacc
    from concourse.masks import make_identity

    offs, nchs, TOT_ALLOC = meta
    NCHSUM = sum(nchs)
    f32 = mybir.dt.float32
    f32r = mybir.dt.float32r
    bf16 = mybir.dt.bfloat16
    f8 = mybir.dt.float8e4
    Act = mybir.ActivationFunctionType
    DR = mybir.MatmulPerfMode.DoubleRow

    nc = bacc.Bacc("TRN2", target_bir_lowering=False, debug=False,
                   num_devices=NCORES)

    # ---- DRAM I/O ----
    xt_d = nc.dram_tensor("xt", [H, TOT_ALLOC], f8, kind="ExternalInput")
    mb_d = nc.dram_tensor("mb", [NCHSUM * 128], f32, kind="ExternalInput")
    cq_d = nc.dram_tensor("cq", [C, H], f32, kind="ExternalInput")
    wdr = {}
    for nm in ("sa_wq", "sa_wk", "sa_wv", "sa_wo", "ca_wq", "ca_wk", "ca_wv",
               "ca_wo"):
        wdr[nm] = nc.dram_tensor(nm, [H, H], f8, kind="ExternalInput")
    bdr = {}
    for nm in ("sa_bq", "sa_bk", "sa_bv", "sa_bo", "ca_bq", "ca_bk", "ca_bv",
               "ca_bo", "ln1_g", "ln1_b", "ln2_g", "ln2_b", "ln3_g", "ln3_b",
               "ff_b2"):
        bdr[nm] = nc.dram_tensor(nm, [H], f32, kind="ExternalInput")
    bias10_d = nc.dram_tensor("bias10", [10, H], bf16, kind="ExternalInput")
    pbias_d = nc.dram_tensor("pbias", [128, 80], f32, kind="ExternalInput")
    w1_d = nc.dram_tensor("ff_w1", [H, FF], bf16, kind="ExternalInput")
    b1_d = nc.dram_tensor("ff_b1", [FF], f32, kind="ExternalInput")
    w2_d = nc.dram_tensor("ff_w2", [FF, H], bf16, kind="ExternalInput")
    wg_d = nc.dram_tensor("wg", [H], f32r, kind="ExternalInput")
    sel2_d = nc.dram_tensor("sel2", [2, 128], f32r, kind="ExternalInput")
    out_d = nc.dram_tensor("out", [NG * C], f32, kind="ExternalOutput")

    def bcast_load(nc, out_ap, dram, nparts, offset=0, inner=H):
        src = bass.AP(tensor=dram.ap().tensor, offset=offset,
                      ap=[[0, nparts], [1, inner]])
        nc.gpsimd.dma_start(out=out_ap, in_=src)

    with tile.TileContext(nc) as tc:
        with (
            tc.tile_pool(name="const", bufs=1) as cp,
            tc.tile_pool(name="ps_mm", bufs=2, space="PSUM") as ps_mm,
            tc.tile_pool(name="dram", bufs=2, space="DRAM") as drp,
        ):
            ident = cp.tile([128, 128], f32)
            make_identity(nc, ident[:])
            ones_f = cp.tile([128, 1], f32)
            nc.vector.memset(ones_f[:], 1.0)
            ones_r = cp.tile([128, 1], f32r)
            nc.vector.tensor_copy(ones_r[:], ones_f[:])
            # dual-fp8 LDWEIGHTS needs >=16 cols per k-plane (walrus
            # s3_lw_dual_fp8_restrictions), so the DR ones is 16 wide and
            # only partition 0 of its output is read.
            ones8p = cp.tile([128, 2, 16], f8)
            nc.vector.memset(ones8p[:], 1.0)
            ones8w = cp.tile([128, 16], f8)
            nc.vector.memset(ones8w[:], 1.0)
            ones_b = cp.tile([128, 16], bf16)
            nc.vector.memset(ones_b[:], 1.0)
            sel_eo = []
            for eo in range(2):
                t = cp.tile([1, 128], f32r, tag=f"sel{eo}")
                nc.scalar.dma_start(t[:], sel2_d.ap()[eo, None, :])
                sel_eo.append(t)
            onesrow_f = cp.tile([1, 128], f32)
            nc.vector.memset(onesrow_f[:], 1.0)
            onesrow_r = cp.tile([1, 128], f32r)
            nc.vector.tensor_copy(onesrow_r[:], onesrow_f[:])
            eps_t = cp.tile([128, 1], f32)
            nc.vector.memset(eps_t[:], EPS)

            # per-partition bias tiles, host-packed contiguous [128, 80]
            pb = cp.tile([128, 80], f32)
            nc.scalar.dma_start(pb[:], pbias_d.ap())
            bk_t = pb[:, 0:8]
            bo_t = pb[:, 8:16]
            g3_t = pb[:, 16:24]
            b3_t = pb[:, 24:32]
            b2_t = pb[:, 32:40]
            b1_t = pb[:, 48:80]
            wg_t = cp.tile([128, MT], f32r)
            nc.vector.tensor_copy(wg_t[:], pb[:, 40:48])
            bv_b = cp.tile([128, H], bf16)
            nc.scalar.dma_start(out=bv_b[:], in_=bass.AP(
                tensor=bias10_d.ap().tensor, offset=9 * H,
                ap=[[0, 128], [1, H]]))

            # persistent activations
            x1t = cp.tile([128, KC, C], f32)        # x1 transposed
            qt_eo = cp.tile([128, KC, 2 * C], f8)   # [q_even | q_odd], zero-pad
            nc.vector.memset(qt_eo[:], 0.0)
            x2t = cp.tile([128, MT, NG * C], f32r)  # residual stream T
            sum2_sb = cp.tile([1, NG * C], f32)
            sq2_sb = cp.tile([1, NG * C], f32)

            def ln_row(pool, x, n_p, g_b, b_b, name):
                """LayerNorm on row-layout x [n_p, H] -> new tile."""
                stats = pool.tile([n_p, 2, 6], f32, tag="ln_st")
                for i in range(2):
                    nc.vector.bn_stats(stats[:, i, :], x[:, i * 512:(i + 1) * 512])
                mv = pool.tile([n_p, 2], f32, tag="ln_mv")
                nc.vector.bn_aggr(mv[:], stats[:])
                rstd = pool.tile([n_p, 1], f32, tag="ln_rs")
                nc.scalar.activation(rstd[:], mv[:, 1:2], Act.Sqrt,
                                     bias=eps_t[:n_p])
                nc.vector.reciprocal(rstd[:], rstd[:])
                y = pool.tile([n_p, H], f32, tag="ln_y")
                nc.vector.tensor_scalar(y[:], x[:], scalar1=mv[:, 0:1],
                                        scalar2=rstd[:],
                                        op0=mybir.AluOpType.subtract,
                                        op1=mybir.AluOpType.mult)
                nc.vector.tensor_mul(y[:], y[:], g_b[:])
                nc.vector.tensor_add(y[:], y[:], b_b[:])
                return y

            def transpose_chunks(pool, src, dst_list, n_p=C):
                """PE-transpose src [n_p, H] into dst slices [128, k, n_p]."""
                for k in range(KC):
                    tp = ps_mm.tile([128, 512], f32, tag="acc")
                    nc.tensor.transpose(tp[:, :n_p], src[:, k * 128:(k + 1) * 128],
                                        ident[:n_p, :n_p])
                    for dst, par in dst_list:
                        if par is None:
                            nc.scalar.copy(dst[:, k, :], tp[:, :n_p])
                        elif par == 0:
                            nc.scalar.copy(dst[0:64, k, 0:n_p], tp[0:64, :n_p])
                        else:
                            nc.scalar.copy(dst[64:128, k, n_p:2 * n_p],
                                           tp[64:128, :n_p])

            def load_w8(pool, w_dram, tag="w8", engs=None):
                """Load [H, H] fp8 weight as [128, KC, H]."""
                w_re = w_dram.ap().rearrange("(k p) n -> p k n", p=128)
                w_t = pool.tile([128, KC, H], f8, tag=tag)
                engs = engs or (nc.sync, nc.gpsimd)
                for i, k2 in enumerate(range(0, KC, 2)):
                    eng = engs[i % len(engs)]
                    eng.dma_start(w_t[:, k2:k2 + 2, :], w_re[:, k2:k2 + 2, :])
                return w_t

            def proj_dr(pool, yt8, w8, bias_b, name, out_dt=f32):
                """fp8-DR projection: out [C, H] = y @ W + b (row layout)."""
                o = pool.tile([C, H], out_dt, tag=f"{name}_o")
                for n in range(2):
                    acc = ps_mm.tile([128, 512], f32, tag="acc")
                    for kp in range(KP):
                        nc.tensor.matmul(acc[:C, :], yt8[:, 2 * kp:2 * kp + 2, :],
                                         w8[:, 2 * kp:2 * kp + 2,
                                            n * 512:(n + 1) * 512],
                                         start=(kp == 0), stop=(kp == KP - 1),
                                         perf_mode=DR)
                    nc.vector.tensor_add(o[:, n * 512:(n + 1) * 512],
                                         acc[:C, :], bias_b[:, n * 512:(n + 1) * 512])
                return o

            # ============ STAGE A + PHASE 1 ============
            with tc.tile_pool(name="mid", bufs=1) as midp:
              ots = midp.tile([128, KC, NG, C], f8)  # attn out T, all slots
              with (
                tc.tile_pool(name="ps_st", bufs=2, space="PSUM") as ps_st,
                tc.tile_pool(name="ps_ot", bufs=1, space="PSUM") as ps_ot,
                tc.tile_pool(name="ps_den", bufs=1, space="PSUM") as ps_den,
                tc.tile_pool(name="ps_rdb", bufs=1, space="PSUM") as ps_rdb,
                tc.tile_pool(name="p1", bufs=1) as p1,
                tc.tile_pool(name="vp", bufs=2) as vp,
                tc.tile_pool(name="ptp", bufs=2) as ptp,
              ):
               # phase-1 inputs stream first so the K-projection can start
               # as soon as the tensor queue drains
               mb = p1.tile([128, NCHSUM], f32, tag="mb")
               nc.scalar.dma_start(mb[:], mb_d.ap().rearrange("(p c) -> p c", p=128))
               wkp = tc.alloc_tile_pool(name="wkp", bufs=1)
               wk_t = load_w8(wkp, wdr["ca_wk"], tag="wkc", engs=(nc.sync,))
               xt = p1.tile([128, KC, TOT_ALLOC], f8, tag="xt")
               xt_re = xt_d.ap().rearrange("(k p) n -> p k n", p=128)
               # column-blocked load so the K-projection can start after the
               # first ~1MB block instead of the whole 4MB tensor
               blocks = []
               boff = 0
               while boff < TOT_ALLOC:
                   bw = min(1024, TOT_ALLOC - boff)
                   blocks.append((boff, bw))
                   boff += bw
               for boff, bw in blocks:
                   for k2 in range(0, KC, 2):
                       eng = nc.sync if k2 % 4 == 0 else nc.gpsimd
                       eng.dma_start(xt[:, k2:k2 + 2, boff:boff + bw],
                                     xt_re[:, k2:k2 + 2, boff:boff + bw])
               wv_t = load_w8(p1, wdr["ca_wv"], tag="wvc", engs=(nc.gpsimd,))

               # K-projection for the whole packed column space (dense DR
               # stream, no deps on stage A); wk frees before stage A opens
               kt = p1.tile([128, MT, TOT_ALLOC], f8, tag="kt")
               if True:
                   for boff, bw in blocks:
                       for m in range(MT):
                           off = boff
                           for pc in _pieces(bw):
                               acc = ps_mm.tile([128, 512], f32, tag="acc")
                               for kp in range(KP):
                                   nc.tensor.matmul(
                                       acc[:, :pc],
                                       wk_t[:, 2 * kp:2 * kp + 2,
                                            m * 128:(m + 1) * 128],
                                       xt[:, 2 * kp:2 * kp + 2, off:off + pc],
                                       start=(kp == 0), stop=(kp == KP - 1),
                                       perf_mode=DR)
                               nc.scalar.activation(
                                   kt[:, m, off:off + pc], acc[:, :pc],
                                   Act.Identity, bias=bk_t[:, m:m + 1])
                               off += pc

               wkp.release()
               with (tc.tile_pool(name="sa", bufs=1) as sp,
                     tc.tile_pool(name="wsa", bufs=2) as wsa):
                wq8 = load_w8(wsa, wdr["sa_wq"])
                wk8 = load_w8(wsa, wdr["sa_wk"])
                wv8 = load_w8(wsa, wdr["sa_wv"])
                bb9 = sp.tile([C, 9, H], bf16, tag="bb9")
                nc.scalar.dma_start(out=bb9[:], in_=bass.AP(
                    tensor=bias10_d.ap().tensor, offset=0,
                    ap=[[0, C], [1, 9 * H]]))
                bias_bcast = {}
                for bi, nm in enumerate(("sa_bq", "sa_bk", "sa_bv", "sa_bo",
                                         "ca_bq", "ln1_g", "ln1_b", "ln2_g",
                                         "ln2_b")):
                    bias_bcast[nm] = bb9[:, bi, :]

                x0 = sp.tile([C, H], f32)
                nc.scalar.dma_start(x0[:], cq_d.ap())
                y1 = ln_row(sp, x0, C, bias_bcast["ln1_g"], bias_bcast["ln1_b"], "ln1")
                y1t = sp.tile([128, KC, C], f8)
                transpose_chunks(sp, y1, [(y1t, None)])
                q1 = proj_dr(sp, y1t, wq8, bias_bcast["sa_bq"], "q1")
                k1 = proj_dr(sp, y1t, wk8, bias_bcast["sa_bk"], "k1")
                v1 = proj_dr(sp, y1t, wv8, bias_bcast["sa_bv"], "v1")

                k1t = sp.tile([128, KC, C], bf16)
                transpose_chunks(sp, k1, [(k1t, None)])
                q1t_eo = sp.tile([128, KC, 2 * C], bf16)
                nc.vector.memset(q1t_eo[:], 0.0)
                transpose_chunks(sp, q1, [(q1t_eo, 0), (q1t_eo, 1)])
                v1b = sp.tile([128, NH, DH], bf16)
                nc.vector.memset(v1b[:], 0.0)
                nc.vector.tensor_copy(
                    v1b[0:64, :, :], v1[:].rearrange("p (h d) -> p h d", d=DH))

                # self-attn scores/exp (keys=64, one chunk)
                pt1 = sp.tile([128, NH, C], bf16)
                nc.vector.memset(pt1[:], 0.0)
                for half in range(2):
                    st = ps_st.tile([128, 4, 2 * C], f32, tag="st")
                    for i in range(4):
                        t = half * 4 + i
                        nc.tensor.matmul(st[:C, i, :], k1t[:, t, :],
                                         q1t_eo[:, t, :], start=True, stop=True)
                    nc.scalar.activation(
                        pt1[0:C, half * 8:(half + 1) * 8, :],
                        st[:C, :, :].rearrange("p a b -> p (a b)").rearrange(
                            "p (h c) -> p h c", c=C),
                        Act.Exp, bias=0.0, scale=SCALE)
                dsb = [sp.tile([1, 512], f32r, tag=f"dsb{e}", name=f"dsb_a{e}")
                       for e in range(2)]
                for hf in range(2):
                    den1 = ps_den.tile([16, 512], f32, tag="den")
                    nc.tensor.matmul(
                        den1[:], ones_b[:],
                        pt1[:, hf * 8:(hf + 1) * 8, :].rearrange(
                            "p h c -> p (h c)"),
                        start=True, stop=True)
                    with nc.allow_low_precision(reason="f32r rden for bcast matmul"):
                        nc.vector.reciprocal(dsb[hf][:], den1[0:1, :])
                ot1 = ps_ot.tile([128, KC, 2 * C], f32, tag="ot")
                for t in range(KC):
                    nc.tensor.matmul(
                        ot1[:, t, :],
                        v1b[:, 2 * t:2 * t + 2, :].rearrange("p a d -> p (a d)"),
                        pt1[:, 2 * t:2 * t + 2, :].rearrange("p a c -> p (a c)"),
                        start=True, stop=True)
                # broadcast 1/den across partitions: accumulate
                # sel_e⊗even + sel_o⊗odd heads (full-128 dst, ISA-safe);
                # dsb half hf holds heads 8hf..8hf+7 -> strided eo slice
                rdb1 = ps_rdb.tile([128, KC * C], f32, tag="rdb")
                for hf in range(2):
                    dv = dsb[hf][:].rearrange("p (h c) -> p h c", c=C)
                    for eo in range(2):
                        nc.tensor.matmul(rdb1[:, hf * 256:(hf + 1) * 256],
                                         sel_eo[eo][:], dv[:, eo::2, :],
                                         start=(eo == 0), stop=(eo == 1))
                ot1s = sp.tile([128, KC, C], f8)
                rdb1_sb = sp.tile([128, KC, C], f32, tag="rdb1_sb")
                nc.scalar.copy(rdb1_sb[:].rearrange("p t c -> p (t c)"), rdb1[:])
                nc.vector.tensor_mul(ot1s[0:64], ot1[0:64, :, 0:C], rdb1_sb[0:64])
                nc.vector.tensor_mul(ot1s[64:128], ot1[64:128, :, C:2 * C],
                                     rdb1_sb[64:128])

                # O-proj + residual -> x1 row layout (fp8-DR)
                wo8 = load_w8(wsa, wdr["sa_wo"])
                x1 = sp.tile([C, H], f32)
                for n in range(2):
                    acc = ps_mm.tile([128, 512], f32, tag="acc")
                    for kp in range(KP):
                        nc.tensor.matmul(acc[:C, :], ot1s[:, 2 * kp:2 * kp + 2, :],
                                         wo8[:, 2 * kp:2 * kp + 2,
                                             n * 512:(n + 1) * 512],
                                         start=(kp == 0), stop=(kp == KP - 1),
                                         perf_mode=DR)
                    nc.vector.tensor_add(x1[:, n * 512:(n + 1) * 512], acc[:C, :],
                                         bias_bcast["sa_bo"][:, n * 512:(n + 1) * 512])
                    nc.vector.tensor_add(x1[:, n * 512:(n + 1) * 512],
                                         x1[:, n * 512:(n + 1) * 512],
                                         x0[:, n * 512:(n + 1) * 512])

                y2 = ln_row(sp, x1, C, bias_bcast["ln2_g"], bias_bcast["ln2_b"], "ln2")
                y2t = sp.tile([128, KC, C], f8)
                transpose_chunks(sp, y2, [(y2t, None)])
                wq8c = load_w8(wsa, wdr["ca_wq"])
                qca = proj_dr(sp, y2t, wq8c, bias_bcast["ca_bq"], "q1")
                transpose_chunks(sp, qca, [(qt_eo, 0), (qt_eo, 1)])
                transpose_chunks(sp, x1, [(x1t, None)])

               # ============ PHASE 1: per-slot cross-attention ============
               if True:
                    def compute_v(g):
                        goff, nch = offs[g], nchs[g]
                        v = vp.tile([128, nchs[0], NH, DH], f8, tag="v",
                                    name=f"v_{g}")
                        for ch in range(nch):
                            coff = goff + ch * 128
                            for half in range(2):
                                acc = ps_mm.tile([128, 512], f32, tag="acc")
                                for kp in range(KP):
                                    nc.tensor.matmul(
                                        acc[:],
                                        xt[:, 2 * kp:2 * kp + 2, coff:coff + 128],
                                        wv_t[:, 2 * kp:2 * kp + 2,
                                             half * 512:(half + 1) * 512],
                                        start=(kp == 0), stop=(kp == KP - 1),
                                        perf_mode=DR)
                                nc.vector.tensor_add(
                                    v[:, ch, half * 8:(half + 1) * 8, :],
                                    acc[:].rearrange("p (h d) -> p h d", d=DH),
                                    bv_b[:, half * 512:(half + 1) * 512].rearrange(
                                        "p (h d) -> p h d", d=DH))
                        return v

                    cc0 = 0  # running chunk index into mb
                    for g in range(NG):
                        goff, nch = offs[g], nchs[g]
                        v = compute_v(g)

                        pt = ptp.tile([128, nchs[0], NH, C], f8, tag="pt")
                        for ch in range(nch):
                            coff = goff + ch * 128
                            for half in range(2):
                                st = ps_st.tile([128, 4, 2 * C], f32, tag="st")
                                for i in range(4):
                                    t = half * 4 + i
                                    nc.tensor.matmul(
                                        st[:, i, :],
                                        kt[:, t, coff:coff + 128],
                                        qt_eo[:, t, :],
                                        start=True, stop=True)
                                nc.scalar.activation(
                                    pt[:, ch, half * 8:(half + 1) * 8, :],
                                    st[:].rearrange("p a b -> p (a b)").rearrange(
                                        "p (h c) -> p h c", c=C),
                                    Act.Exp, bias=mb[:, cc0 + ch:cc0 + ch + 1],
                                    scale=SCALE)

                        dsb2 = [p1.tile([1, 512], f32r, tag=f"dsb{e}",
                                        name=f"dsb2_{e}") for e in range(2)]
                        nDR = nch // 2
                        for hf in range(2):
                            hs = slice(hf * 8, (hf + 1) * 8)
                            den = ps_den.tile([16, 512], f32, tag="den")
                            for cp2 in range(nDR):
                                nc.tensor.matmul(
                                    den[:], ones8p[:],
                                    pt[:, 2 * cp2:2 * cp2 + 2, hs, :].rearrange(
                                        "p a h c -> p a (h c)"),
                                    start=(cp2 == 0), stop=(nch % 2 == 0 and
                                                            cp2 == nDR - 1),
                                    perf_mode=DR)
                            if nch % 2:
                                nc.tensor.matmul(
                                    den[:], ones8w[:],
                                    pt[:, nch - 1, hs, :].rearrange(
                                        "p h c -> p (h c)"),
                                    start=(nDR == 0), stop=True)
                            with nc.allow_low_precision(
                                    reason="f32r rden for bcast matmul"):
                                nc.vector.reciprocal(dsb2[hf][:], den[0:1, :])
                        ot = ps_ot.tile([128, KC, 2 * C], f32, tag="ot")
                        for t in range(KC):
                            for cp2 in range(nDR):
                                nc.tensor.matmul(
                                    ot[:, t, :],
                                    v[:, 2 * cp2:2 * cp2 + 2, 2 * t:2 * t + 2,
                                      :].rearrange("p a b d -> p a (b d)"),
                                    pt[:, 2 * cp2:2 * cp2 + 2, 2 * t:2 * t + 2,
                                       :].rearrange("p a b c -> p a (b c)"),
                                    start=(cp2 == 0), stop=(nch % 2 == 0 and
                                                            cp2 == nDR - 1),
                                    perf_mode=DR)
                            if nch % 2:
                                nc.tensor.matmul(
                                    ot[:, t, :],
                                    v[:, nch - 1, 2 * t:2 * t + 2, :].rearrange(
                                        "p a d -> p (a d)"),
                                    pt[:, nch - 1, 2 * t:2 * t + 2, :].rearrange(
                                        "p a c -> p (a c)"),
                                    start=(nDR == 0), stop=True)
                        rdb = ps_rdb.tile([128, KC * C], f32, tag="rdb")
                        for hf in range(2):
                            dv = dsb2[hf][:].rearrange("p (h c) -> p h c", c=C)
                            for eo in range(2):
                                nc.tensor.matmul(
                                    rdb[:, hf * 256:(hf + 1) * 256],
                                    sel_eo[eo][:], dv[:, eo::2, :],
                                    start=(eo == 0), stop=(eo == 1))
                        rdb_sb = p1.tile([128, KC, C], f32, tag="rdb_sb")
                        nc.scalar.copy(rdb_sb[:].rearrange("p t c -> p (t c)"),
                                       rdb[:])
                        nc.vector.tensor_mul(ots[0:64, :, g, :],
                                             ot[0:64, :, 0:C], rdb_sb[0:64])
                        nc.vector.tensor_mul(ots[64:128, :, g, :],
                                             ot[64:128, :, C:2 * C],
                                             rdb_sb[64:128])
                        cc0 += nch

              # ============ PHASE 2a: O-projection (fp8-DR) ============
              with (tc.tile_pool(name="wop", bufs=1) as wop,
                    tc.tile_pool(name="sq0p", bufs=2) as sq0p,
                    tc.tile_pool(name="ps_st0", bufs=2, space="PSUM") as ps_st0):
                sum_ps0 = ps_st0.tile([1, NG * C], f32, tag="stat0")
                sq_ps0 = ps_st0.tile([1, NG * C], f32, tag="stat0")
                wo_t = load_w8(wop, wdr["ca_wo"], tag="woc")
                for m in range(MT):
                    acc = ps_mm.tile([128, 512], f32, tag="acc")
                    for kp in range(KP):
                        nc.tensor.matmul(
                            acc[:],
                            wo_t[:, 2 * kp:2 * kp + 2, m * 128:(m + 1) * 128],
                            ots[:, 2 * kp:2 * kp + 2, :, :].rearrange(
                                "p a g c -> p a (g c)"),
                            start=(kp == 0), stop=(kp == KP - 1),
                            perf_mode=DR)
                    nc.scalar.activation(x2t[:, m, :], acc[:], Act.Identity,
                                         bias=bo_t[:, m:m + 1])
                    nc.vector.tensor_add(
                        x2t[:, m, :].rearrange("p (g c) -> p g c", c=C),
                        x2t[:, m, :].rearrange("p (g c) -> p g c", c=C),
                        x1t[:, m, None, :].to_broadcast((128, NG, C)))
                    sq0 = sq0p.tile([128, NG * C], f32r, tag="sq0")
                    nc.vector.tensor_mul(sq0[:], x2t[:, m, :], x2t[:, m, :])
                    nc.tensor.matmul(sum_ps0[:], ones_r[:], x2t[:, m, :],
                                     start=(m == 0), stop=(m == MT - 1))
                    nc.tensor.matmul(sq_ps0[:], ones_r[:], sq0[:],
                                     start=(m == 0), stop=(m == MT - 1))
                nc.vector.tensor_copy(sum2_sb[:], sum_ps0[:])
                nc.vector.tensor_copy(sq2_sb[:], sq_ps0[:])

            # ============ PHASE 2: FFN (bf16), output ============
            with (
                tc.tile_pool(name="p2", bufs=1) as p2,
                tc.tile_pool(name="wstr", bufs=3) as wstr,
                tc.tile_pool(name="wstr2", bufs=2) as wstr2,
                tc.tile_pool(name="sq", bufs=2) as sqp,
                tc.tile_pool(name="ps_stat", bufs=4, space="PSUM") as ps_stat,
                tc.tile_pool(name="ps_bc", bufs=2, space="PSUM") as ps_bc,
            ):
                R = NG * C  # 512 rows
                # LN3 stats were accumulated during O-proj (sum2_sb/sq2_sb)
                mean = p2.tile([1, R], f32r, tag="mean")
                with nc.allow_low_precision(reason="f32r mean/rstd for K=1 bcast matmul"):
                    nc.scalar.mul(mean[:], sum2_sb[:], 1.0 / H)
                var = p2.tile([1, R], f32, tag="var")
                nc.scalar.mul(var[:], sq2_sb[:], 1.0 / H)
                m2 = p2.tile([1, R], f32, tag="m2")
                nc.vector.tensor_mul(m2[:], mean[:], mean[:])
                nc.vector.tensor_sub(var[:], var[:], m2[:])
                rstd = p2.tile([1, R], f32r, tag="rstd")
                with nc.allow_low_precision(reason="f32r mean/rstd for K=1 bcast matmul"):
                    nc.scalar.activation(rstd[:], var[:], Act.Sqrt, bias=eps_t[0:1])
                    nc.vector.reciprocal(rstd[:], rstd[:])
                mean_b = ps_bc.tile([128, R], f32, tag="bc")
                rstd_b = ps_bc.tile([128, R], f32, tag="bc")
                nc.tensor.matmul(mean_b[:], onesrow_r[:], mean[:],
                                 start=True, stop=True)
                nc.tensor.matmul(rstd_b[:], onesrow_r[:], rstd[:],
                                 start=True, stop=True)

                y3t = p2.tile([128, KC, R], bf16, tag="y3t")
                for m in range(MT):
                    nc.vector.tensor_sub(y3t[:, m, :], x2t[:, m, :], mean_b[:])
                    nc.vector.tensor_mul(y3t[:, m, :], y3t[:, m, :], rstd_b[:])
                    nc.vector.tensor_scalar(
                        y3t[:, m, :], y3t[:, m, :],
                        scalar1=g3_t[:, m:m + 1], scalar2=b3_t[:, m:m + 1],
                        op0=mybir.AluOpType.mult, op1=mybir.AluOpType.add)

                # GEMM1: h1T [128, FM, R] bf16
                h1t = p2.tile([128, FM, R], bf16, tag="h1t")
                w1_re = w1_d.ap().rearrange("(k p) f -> p k f", p=128)
                for fm in range(FM):
                    w1c = wstr.tile([128, KC, 128], bf16, tag="w1c")
                    for k4 in range(0, KC, 4):
                        nc.sync.dma_start(w1c[:, k4:k4 + 4, :],
                                          w1_re[:, k4:k4 + 4, fm * 128:(fm + 1) * 128])
                    acc = ps_mm.tile([128, 512], f32, tag="acc")
                    for k in range(KC):
                        nc.tensor.matmul(acc[:], w1c[:, k, :], y3t[:, k, :],
                                         start=(k == 0), stop=(k == KC - 1))
                    nc.scalar.activation(h1t[:, fm, :], acc[:], Act.Relu,
                                         bias=b1_t[:, fm:fm + 1])

                # GEMM2: x3T = W2^T-chunks @ h1T + x2T + b2
                sum3 = ps_stat.tile([1, R], f32, tag="stat")
                sq3 = ps_stat.tile([1, R], f32, tag="stat")
                a_ps = ps_stat.tile([1, R], f32, tag="stat")
                x3t = p2.tile([128, MT, R], f32r, tag="x3t")
                w2_re = w2_d.ap().rearrange("(k p) f -> p k f", p=128)
                w2cs = {}

                def load_w2c(m):
                    w2c = wstr2.tile([128, FM, 128], bf16, tag="w2c")
                    for f8_ in range(0, FM, 8):
                        nc.gpsimd.dma_start(
                            w2c[:, f8_:f8_ + 8, :],
                            w2_re[:, f8_:f8_ + 8, m * 128:(m + 1) * 128])
                    w2cs[m] = w2c
                load_w2c(0)
                for m in range(MT):
                    w2c = w2cs.pop(m)
                    if m + 1 < MT:
                        load_w2c(m + 1)
                    acc = ps_mm.tile([128, 512], f32, tag="acc")
                    for fk in range(FM):
                        nc.tensor.matmul(acc[:], w2c[:, fk, :], h1t[:, fk, :],
                                         start=(fk == 0), stop=(fk == FM - 1))
                    nc.scalar.activation(x3t[:, m, :], acc[:], Act.Identity,
                                         bias=b2_t[:, m:m + 1])
                    nc.vector.tensor_add(x3t[:, m, :], x3t[:, m, :],
                                         x2t[:, m, :])
                    sq = sqp.tile([128, R], f32r, tag="sq")
                    nc.vector.tensor_mul(sq[:], x3t[:, m, :], x3t[:, m, :])
                    nc.tensor.matmul(sum3[:], ones_r[:], x3t[:, m, :],
                                     start=(m == 0), stop=(m == MT - 1))
                    nc.tensor.matmul(sq3[:], ones_r[:], sq[:],
                                     start=(m == 0), stop=(m == MT - 1))
                    nc.tensor.matmul(a_ps[:], wg_t[:, m:m + 1], x3t[:, m, :],
                                     start=(m == 0), stop=(m == MT - 1))

                # final LN + linear folded: logits = rstd*(A - mean*S1) + S2
                mean3 = p2.tile([1, R], f32, tag="mean3")
                nc.scalar.mul(mean3[:], sum3[:], 1.0 / H)
                var3 = p2.tile([1, R], f32, tag="var3")
                nc.scalar.mul(var3[:], sq3[:], 1.0 / H)
                m23 = p2.tile([1, R], f32, tag="m23")
                nc.vector.tensor_mul(m23[:], mean3[:], mean3[:])
                nc.vector.tensor_sub(var3[:], var3[:], m23[:])
                rstd3 = p2.tile([1, R], f32, tag="rstd3")
                nc.scalar.activation(rstd3[:], var3[:], Act.Sqrt, bias=eps_t[0:1])
                nc.vector.reciprocal(rstd3[:], rstd3[:])
                logits = p2.tile([1, R], f32, tag="logits")
                nc.scalar.mul(logits[:], mean3[:], -S1)
                nc.vector.tensor_add(logits[:], logits[:], a_ps[:])
                nc.vector.tensor_mul(logits[:], logits[:], rstd3[:])
                nc.scalar.add(logits[:], logits[:], S2)
                nc.sync.dma_start(out_d.ap()[None, :], logits[:])

    nc.compile()
    return nc


def _ceil16(n):
    return -(-int(n) // 16) * 16


def _prep(inputs):
    nf = np.ascontiguousarray(np.asarray(inputs["node_features"], np.float32))
    batch = np.asarray(inputs["batch"]).astype(np.int64)
    counts = np.bincount(batch, minlength=B)
    offsets = np.concatenate([[0], np.cumsum(counts)[:-1]])

    # slot packing: rank graphs by count desc; slot j holds ranks [8j, 8j+8)
    order = np.argsort(-counts, kind="stable")
    L = [_ceil16(max(int(counts[order[j * NCORES]]), 16)) for j in range(NG)]
    offs = np.concatenate([[0], np.cumsum(L)[:-1]]).astype(int).tolist()
    nchs = [-(-l // 128) for l in L]
    TOT_ALLOC = _ceil16(offs[-1] + nchs[-1] * 128)
    meta = (offs, nchs, TOT_ALLOC)

    xts, mbs = [], []
    NCHSUM = sum(nchs)
    for c in range(NCORES):
        xt = np.zeros((H, TOT_ALLOC), np.float32)
        mbm = np.full((NCHSUM, 128), NEG, np.float32)
        cc0 = 0
        for j in range(NG):
            g = int(order[j * NCORES + c])
            n = int(counts[g])
            o = int(offsets[g])
            xt[:, offs[j]:offs[j] + n] = nf[o:o + n].T
            # empty graph guard: unmask one zero-feature key
            nv = max(n, 1)
            for ch in range(nchs[j]):
                lo = ch * 128
                mbm[cc0 + ch, :max(0, min(nv - lo, 128))] = 0.0
            cc0 += nchs[j]
        xts.append(np.clip(xt, -240.0, 240.0).astype(F8))
        mbs.append(np.ascontiguousarray(mbm.T.reshape(-1)))

    out_w = np.asarray(inputs["out_w"], np.float32)[:, 0]
    og = np.asarray(inputs["out_ln_g"], np.float32)
    ob = np.asarray(inputs["out_ln_b"], np.float32)
    wg = (out_w * og).astype(np.float32)
    S1 = float(wg.sum())
    S2 = float((out_w * ob).sum() + np.asarray(inputs["out_b"], np.float32)[0])

    def perm128(v):
        return np.asarray(v, np.float32).reshape(-1, 128).T

    pbias = np.zeros((128, 80), np.float32)
    pbias[:, 0:8] = perm128(inputs["ca_bk"])
    pbias[:, 8:16] = perm128(inputs["ca_bo"])
    pbias[:, 16:24] = perm128(inputs["ln3_g"])
    pbias[:, 24:32] = perm128(inputs["ln3_b"])
    pbias[:, 32:40] = perm128(inputs["ff_b2"])
    pbias[:, 40:48] = perm128(wg)
    pbias[:, 48:80] = perm128(inputs["ff_b1"])
    sel2 = np.zeros((2, 128), np.float32)
    sel2[0, 0:64] = 1.0
    sel2[1, 64:128] = 1.0
    common = {"cq": np.ascontiguousarray(np.asarray(inputs["class_queries"], np.float32)),
              "wg": wg, "sel2": sel2, "pbias": pbias}
    for nm in ("sa_wq", "sa_wk", "sa_wv", "sa_wo", "ca_wq", "ca_wk", "ca_wv",
               "ca_wo"):
        common[nm] = np.clip(np.asarray(inputs[nm], np.float32),
                             -240.0, 240.0).astype(F8)
    for nm in ("ff_w1", "ff_w2"):
        common[nm] = np.asarray(inputs[nm], np.float32).astype(BF16)
    for nm in ("sa_bq", "sa_bk", "sa_bv", "sa_bo", "ca_bq", "ca_bk",
               "ca_bv", "ca_bo", "ln1_g", "ln1_b", "ln2_g", "ln2_b",
               "ln3_g", "ln3_b", "ff_b1", "ff_b2"):
        common[nm] = np.ascontiguousarray(np.asarray(inputs[nm], np.float32))
    common["bias10"] = np.stack(
        [np.asarray(inputs[nm], np.float32) for nm in
         ("sa_bq", "sa_bk", "sa_bv", "sa_bo", "ca_bq",
          "ln1_g", "ln1_b", "ln2_g", "ln2_b", "ca_bv")]).astype(BF16)

    in_maps = []
    for c in range(NCORES):
        m = dict(common)
        m["xt"] = xts[c]
        m["mb"] = mbs[c]
        in_maps.append(m)
    return meta, S1, S2, in_maps, order


def _run(inputs, trace=False):
    from concourse.bass_utils import run_bass_kernel_spmd
    meta, S1, S2, in_maps, order = _prep(inputs)
    nc = build_nc(meta, S1, S2)
    try:
        r = run_bass_kernel_spmd(nc, in_maps, core_ids=list(range(NCORES)),
                                 trace=trace)
    except Exception:
        # transient device wedge (NRT_EXEC_UNIT_UNRECOVERABLE) clears on retry
        r = run_bass_kernel_spmd(nc, in_maps, core_ids=list(range(NCORES)),
                                 trace=trace)
    out = np.zeros((B, C), np.float32)
    for c in range(NCORES):
        rc = r.results[c]["out"].reshape(NG, C)
        for j in range(NG):
            out[int(order[j * NCORES + c])] = rc[j]
    return out.astype(np.float32), r


def kernel(**inputs):
    return _run(inputs, trace=False)[0]


# revision 36
# speedup vs baseline: 1.0341x; 1.0109x over previous
"""Trainium2 Bass kernel for nn_ClassQueryHead (transformer decoder head over
ragged graph batches).

Strategy: data-parallel over graphs (8 graphs per core x 8 cores), with
size-balanced slot packing: graphs sorted by node count, slot j on every core
holds one of ranks [8j, 8j+8), padded to L_j = ceil16(max count in slot) --
~4080 key columns per core instead of 8*640.

Device pipeline per core:
  stage A (shared): self-attn block on class queries, fp8-DoubleRow
    projections (weights+activations e4m3, fp32 accumulate)
  phase 1: K projection for the whole core's packed columns (fp8-DR),
    then per slot: V projection (fp8-DR), scores (fp8 matmul vs shared
    query tile), exp fused on ACT with the ragged mask as per-partition
    bias, denominator via fp8-DR ones-matmul, PV (fp8 matmul)
  phase 2: O-projection (fp8-DR) into the transposed residual stream,
    LayerNorm via ones-matmul partition reductions, FFN in bf16 (fp8
    breaks the 2e-2 error budget), final LN+Linear folded into one
    matmul with wg = out_w * ln_g.
"""
import numpy as np
import ml_dtypes

H = 1024
NH = 16
DH = 64
C = 64
B = 64
FF = 4096
EPS = 1e-5
SCALE = 0.125
NCORES = 8
NG = B // NCORES  # graphs (slots) per core
KC = H // 128     # contract chunks of H
KP = KC // 2      # fp8 DoubleRow pairs
MT = H // 128     # m tiles of H
FM = FF // 128    # ff tiles
NEG = -1e30
F8 = ml_dtypes.float8_e4m3
BF16 = ml_dtypes.bfloat16


def _pieces(n):
    """Split free dim n into pieces <=512."""
    out = []
    rem = n
    while rem > 512:
        out.append(512)
        rem -= 512
    if rem:
        out.append(rem)
    return out


def build_nc(meta, S1, S2):
    import concourse.bass as bass
    import concourse.tile as tile
    import concourse.mybir as mybir
    from concourse import bacc
    from concourse.masks import make_identity

    offs, nchs, TOT_ALLOC = meta
    NCHSUM = sum(nchs)
    f32 = mybir.dt.float32
    f32r = mybir.dt.float32r
    bf16 = mybir.dt.bfloat16
    f8 = mybir.dt.float8e4
    Act = mybir.ActivationFunctionType
    DR = mybir.MatmulPerfMode.DoubleRow

    nc = bacc.Bacc("TRN2", target_bir_lowering=False, debug=False,
                   num_devices=NCORES)

    # ---- DRAM I/O ----
    xt_d = nc.dram_tensor("xt", [H, TOT_ALLOC], f8, kind="ExternalInput")
    mb_d = nc.dram_tensor("mb", [NCHSUM * 128], f32, kind="ExternalInput")
    cq_d = nc.dram_tensor("cq", [C, H], f32, kind="ExternalInput")
    wdr = {}
    for nm in ("sa_wq", "sa_wk", "sa_wv", "sa_wo", "ca_wq", "ca_wk", "ca_wv",
               "ca_wo"):
        wdr[nm] = nc.dram_tensor(nm, [H, H], f8, kind="ExternalInput")
    bdr = {}
    for nm in ("sa_bq", "sa_bk", "sa_bv", "sa_bo", "ca_bq", "ca_bk", "ca_bv",
               "ca_bo", "ln1_g", "ln1_b", "ln2_g", "ln2_b", "ln3_g", "ln3_b",
               "ff_b2"):
        bdr[nm] = nc.dram_tensor(nm, [H], f32, kind="ExternalInput")
    bias10_d = nc.dram_tensor("bias10", [10, H], bf16, kind="ExternalInput")
    pbias_d = nc.dram_tensor("pbias", [128, 80], f32, kind="ExternalInput")
    w1_d = nc.dram_tensor("ff_w1", [H, FF], bf16, kind="ExternalInput")
    b1_d = nc.dram_tensor("ff_b1", [FF], f32, kind="ExternalInput")
    w2_d = nc.dram_tensor("ff_w2", [FF, H], bf16, kind="ExternalInput")
    wg_d = nc.dram_tensor("wg", [H], f32r, kind="ExternalInput")
    sel2_d = nc.dram_tensor("sel2", [2, 128], f32r, kind="ExternalInput")
    out_d = nc.dram_tensor("out", [NG * C], f32, kind="ExternalOutput")

    def bcast_load(nc, out_ap, dram, nparts, offset=0, inner=H):
        src = bass.AP(tensor=dram.ap().tensor, offset=offset,
                      ap=[[0, nparts], [1, inner]])
        nc.gpsimd.dma_start(out=out_ap, in_=src)

    with tile.TileContext(nc) as tc:
        with (
            tc.tile_pool(name="const", bufs=1) as cp,
            tc.tile_pool(name="ps_mm", bufs=2, space="PSUM") as ps_mm,
            tc.tile_pool(name="dram", bufs=2, space="DRAM") as drp,
        ):
            ident = cp.tile([128, 128], f32)
            make_identity(nc, ident[:])
            ones_f = cp.tile([128, 1], f32)
            nc.vector.memset(ones_f[:], 1.0)
            ones_r = cp.tile([128, 1], f32r)
            nc.vector.tensor_copy(ones_r[:], ones_f[:])
            # dual-fp8 LDWEIGHTS needs >=16 cols per k-plane (walrus
            # s3_lw_dual_fp8_restrictions), so the DR ones is 16 wide and
            # only partition 0 of its output is read.
            ones8p = cp.tile([128, 2, 16], f8)
            nc.vector.memset(ones8p[:], 1.0)
            ones8w = cp.tile([128, 16], f8)
            nc.vector.memset(ones8w[:], 1.0)
            ones_b = cp.tile([128, 16], bf16)
            nc.vector.memset(ones_b[:], 1.0)
            sel_eo = []
            for eo in range(2):
                t = cp.tile([1, 128], f32r, tag=f"sel{eo}")
                nc.scalar.dma_start(t[:], sel2_d.ap()[eo, None, :])
                sel_eo.append(t)
            onesrow_f = cp.tile([1, 128], f32)
            nc.vector.memset(onesrow_f[:], 1.0)
            onesrow_r = cp.tile([1, 128], f32r)
            nc.vector.tensor_copy(onesrow_r[:], onesrow_f[:])
            eps_t = cp.tile([128, 1], f32)
            nc.vector.memset(eps_t[:], EPS)

            # per-partition bias tiles, host-packed contiguous [128, 80]
            pb = cp.tile([128, 80], f32)
            nc.scalar.dma_start(pb[:], pbias_d.ap())
            bk_t = pb[:, 0:8]
            bo_t = pb[:, 8:16]
            g3_t = pb[:, 16:24]
            b3_t = pb[:, 24:32]
            b2_t = pb[:, 32:40]
            b1_t = pb[:, 48:80]
            wg_t = cp.tile([128, MT], f32r)
            nc.vector.tensor_copy(wg_t[:], pb[:, 40:48])
            bv_b = cp.tile([128, H], bf16)
            nc.scalar.dma_start(out=bv_b[:], in_=bass.AP(
                tensor=bias10_d.ap().tensor, offset=9 * H,
                ap=[[0, 128], [1, H]]))

            # persistent activations
            x1t = cp.tile([128, KC, C], f32)        # x1 transposed
            qt_eo = cp.tile([128, KC, 2 * C], f8)   # [q_even | q_odd], zero-pad
            nc.vector.memset(qt_eo[:], 0.0)
            x2t = cp.tile([128, MT, NG * C], f32r)  # residual stream T
            sum2_sb = cp.tile([1, NG * C], f32)
            sq2_sb = cp.tile([1, NG * C], f32)

            def ln_row(pool, x, n_p, g_b, b_b, name):
                """LayerNorm on row-layout x [n_p, H] -> new tile."""
                stats = pool.tile([n_p, 2, 6], f32, tag="ln_st")
                for i in range(2):
                    nc.vector.bn_stats(stats[:, i, :], x[:, i * 512:(i + 1) * 512])
                mv = pool.tile([n_p, 2], f32, tag="ln_mv")
                nc.vector.bn_aggr(mv[:], stats[:])
                rstd = pool.tile([n_p, 1], f32, tag="ln_rs")
                nc.scalar.activation(rstd[:], mv[:, 1:2], Act.Sqrt,
                                     bias=eps_t[:n_p])
                nc.vector.reciprocal(rstd[:], rstd[:])
                y = pool.tile([n_p, H], f32, tag="ln_y")
                nc.vector.tensor_scalar(y[:], x[:], scalar1=mv[:, 0:1],
                                        scalar2=rstd[:],
                                        op0=mybir.AluOpType.subtract,
                                        op1=mybir.AluOpType.mult)
                nc.vector.tensor_mul(y[:], y[:], g_b[:])
                nc.vector.tensor_add(y[:], y[:], b_b[:])
                return y

            def transpose_chunks(pool, src, dst_list, n_p=C):
                """PE-transpose src [n_p, H] into dst slices [128, k, n_p]."""
                for k in range(KC):
                    tp = ps_mm.tile([128, 512], f32, tag="acc")
                    nc.tensor.transpose(tp[:, :n_p], src[:, k * 128:(k + 1) * 128],
                                        ident[:n_p, :n_p])
                    for dst, par in dst_list:
                        if par is None:
                            nc.scalar.copy(dst[:, k, :], tp[:, :n_p])
                        elif par == 0:
                            nc.scalar.copy(dst[0:64, k, 0:n_p], tp[0:64, :n_p])
                        else:
                            nc.scalar.copy(dst[64:128, k, n_p:2 * n_p],
                                           tp[64:128, :n_p])

            def load_w8(pool, w_dram, tag="w8", engs=None):
                """Load [H, H] fp8 weight as [128, KC, H]."""
                w_re = w_dram.ap().rearrange("(k p) n -> p k n", p=128)
                w_t = pool.tile([128, KC, H], f8, tag=tag)
                engs = engs or (nc.sync, nc.gpsimd)
                for i, k2 in enumerate(range(0, KC, 2)):
                    eng = engs[i % len(engs)]
                    eng.dma_start(w_t[:, k2:k2 + 2, :], w_re[:, k2:k2 + 2, :])
                return w_t

            def proj_dr(pool, yt8, w8, bias_b, name, out_dt=f32):
                """fp8-DR projection: out [C, H] = y @ W + b (row layout)."""
                o = pool.tile([C, H], out_dt, tag=f"{name}_o")
                for n in range(2):
                    acc = ps_mm.tile([128, 512], f32, tag="acc")
                    for kp in range(KP):
                        nc.tensor.matmul(acc[:C, :], yt8[:, 2 * kp:2 * kp + 2, :],
                                         w8[:, 2 * kp:2 * kp + 2,
                                            n * 512:(n + 1) * 512],
                                         start=(kp == 0), stop=(kp == KP - 1),
                                         perf_mode=DR)
                    nc.vector.tensor_add(o[:, n * 512:(n + 1) * 512],
                                         acc[:C, :], bias_b[:, n * 512:(n + 1) * 512])
                return o

            # ============ STAGE A + PHASE 1 ============
            with tc.tile_pool(name="mid", bufs=1) as midp:
              ots = midp.tile([128, KC, NG, C], f8)  # attn out T, all slots
              with (
                tc.tile_pool(name="ps_st", bufs=2, space="PSUM") as ps_st,
                tc.tile_pool(name="ps_ot", bufs=1, space="PSUM") as ps_ot,
                tc.tile_pool(name="ps_den", bufs=1, space="PSUM") as ps_den,
                tc.tile_pool(name="ps_rdb", bufs=1, space="PSUM") as ps_rdb,
                tc.tile_pool(name="p1", bufs=1) as p1,
                tc.tile_pool(name="vp", bufs=2) as vp,
                tc.tile_pool(name="ptp", bufs=2) as ptp,
              ):
               # phase-1 inputs stream first so the K-projection can start
               # as soon as the tensor queue drains
               mb = p1.tile([128, NCHSUM], f32, tag="mb")
               nc.scalar.dma_start(mb[:], mb_d.ap().rearrange("(p c) -> p c", p=128))
               wkp = tc.alloc_tile_pool(name="wkp", bufs=1)
               wk_t = load_w8(wkp, wdr["ca_wk"], tag="wkc", engs=(nc.sync,))
               xt = p1.tile([128, KC, TOT_ALLOC], f8, tag="xt")
               xt_re = xt_d.ap().rearrange("(k p) n -> p k n", p=128)
               # column-blocked load so the K-projection can start after the
               # first ~1MB block instead of the whole 4MB tensor
               blocks = []
               boff = 0
               while boff < TOT_ALLOC:
                   bw = min(1024, TOT_ALLOC - boff)
                   blocks.append((boff, bw))
                   boff += bw
               for boff, bw in blocks:
                   for k2 in range(0, KC, 2):
                       eng = nc.sync if k2 % 4 == 0 else nc.gpsimd
                       eng.dma_start(xt[:, k2:k2 + 2, boff:boff + bw],
                                     xt_re[:, k2:k2 + 2, boff:boff + bw])
               wv_t = load_w8(p1, wdr["ca_wv"], tag="wvc", engs=(nc.gpsimd,))

               # K-projection for the whole packed column space (dense DR
               # stream, no deps on stage A); wk frees before stage A opens
               kt = p1.tile([128, MT, TOT_ALLOC], f8, tag="kt")
               if True:
                   for boff, bw in blocks:
                       for m in range(MT):
                           off = boff
                           for pc in _pieces(bw):
                               acc = ps_mm.tile([128, 512], f32, tag="acc")
                               for kp in range(KP):
                                   nc.tensor.matmul(
                                       acc[:, :pc],
                                       wk_t[:, 2 * kp:2 * kp + 2,
                                            m * 128:(m + 1) * 128],
                                       xt[:, 2 * kp:2 * kp + 2, off:off + pc],
                                       start=(kp == 0), stop=(kp == KP - 1),
                                       perf_mode=DR)
                               nc.scalar.activation(
                                   kt[:, m, off:off + pc], acc[:, :pc],
                                   Act.Identity, bias=bk_t[:, m:m + 1])
                               off += pc

               wkp.release()
               with (tc.tile_pool(name="sa", bufs=1) as sp,
                     tc.tile_pool(name="wsa", bufs=2) as wsa):
                wq8 = load_w8(wsa, wdr["sa_wq"])
                wk8 = load_w8(wsa, wdr["sa_wk"])
                wv8 = load_w8(wsa, wdr["sa_wv"])
                bias_bcast = {}
                for bi, nm in enumerate(("sa_bq", "sa_bk", "sa_bv", "sa_bo",
                                         "ca_bq", "ln1_g", "ln1_b", "ln2_g",
                                         "ln2_b")):
                    t = sp.tile([C, H], bf16, tag=f"bb_{nm}")
                    bsrc = bass.AP(tensor=bias10_d.ap().tensor, offset=bi * H,
                                   ap=[[0, C], [1, H]])
                    nc.scalar.dma_start(out=t[:], in_=bsrc)
                    bias_bcast[nm] = t

                x0 = sp.tile([C, H], f32)
                nc.scalar.dma_start(x0[:], cq_d.ap())
                y1 = ln_row(sp, x0, C, bias_bcast["ln1_g"], bias_bcast["ln1_b"], "ln1")
                y1t = sp.tile([128, KC, C], f8)
                transpose_chunks(sp, y1, [(y1t, None)])
                q1 = proj_dr(sp, y1t, wq8, bias_bcast["sa_bq"], "q1")
                k1 = proj_dr(sp, y1t, wk8, bias_bcast["sa_bk"], "k1")
                v1 = proj_dr(sp, y1t, wv8, bias_bcast["sa_bv"], "v1")

                k1t = sp.tile([128, KC, C], bf16)
                transpose_chunks(sp, k1, [(k1t, None)])
                q1t_eo = sp.tile([128, KC, 2 * C], bf16)
                nc.vector.memset(q1t_eo[:], 0.0)
                transpose_chunks(sp, q1, [(q1t_eo, 0), (q1t_eo, 1)])
                v1b = sp.tile([128, NH, DH], bf16)
                nc.vector.memset(v1b[:], 0.0)
                nc.vector.tensor_copy(
                    v1b[0:64, :, :], v1[:].rearrange("p (h d) -> p h d", d=DH))

                # self-attn scores/exp (keys=64, one chunk)
                pt1 = sp.tile([128, NH, C], bf16)
                nc.vector.memset(pt1[:], 0.0)
                for half in range(2):
                    st = ps_st.tile([128, 4, 2 * C], f32, tag="st")
                    for i in range(4):
                        t = half * 4 + i
                        nc.tensor.matmul(st[:C, i, :], k1t[:, t, :],
                                         q1t_eo[:, t, :], start=True, stop=True)
                    nc.scalar.activation(
                        pt1[0:C, half * 8:(half + 1) * 8, :],
                        st[:C, :, :].rearrange("p a b -> p (a b)").rearrange(
                            "p (h c) -> p h c", c=C),
                        Act.Exp, bias=0.0, scale=SCALE)
                dsb = [sp.tile([1, 512], f32r, tag=f"dsb{e}", name=f"dsb_a{e}")
                       for e in range(2)]
                for hf in range(2):
                    den1 = ps_den.tile([16, 512], f32, tag="den")
                    nc.tensor.matmul(
                        den1[:], ones_b[:],
                        pt1[:, hf * 8:(hf + 1) * 8, :].rearrange(
                            "p h c -> p (h c)"),
                        start=True, stop=True)
                    with nc.allow_low_precision(reason="f32r rden for bcast matmul"):
                        nc.vector.reciprocal(dsb[hf][:], den1[0:1, :])
                ot1 = ps_ot.tile([128, KC, 2 * C], f32, tag="ot")
                for t in range(KC):
                    nc.tensor.matmul(
                        ot1[:, t, :],
                        v1b[:, 2 * t:2 * t + 2, :].rearrange("p a d -> p (a d)"),
                        pt1[:, 2 * t:2 * t + 2, :].rearrange("p a c -> p (a c)"),
                        start=True, stop=True)
                # broadcast 1/den across partitions: accumulate
                # sel_e⊗even + sel_o⊗odd heads (full-128 dst, ISA-safe);
                # dsb half hf holds heads 8hf..8hf+7 -> strided eo slice
                rdb1 = ps_rdb.tile([128, KC * C], f32, tag="rdb")
                for hf in range(2):
                    dv = dsb[hf][:].rearrange("p (h c) -> p h c", c=C)
                    for eo in range(2):
                        nc.tensor.matmul(rdb1[:, hf * 256:(hf + 1) * 256],
                                         sel_eo[eo][:], dv[:, eo::2, :],
                                         start=(eo == 0), stop=(eo == 1))
                ot1s = sp.tile([128, KC, C], f8)
                rdb1_sb = sp.tile([128, KC, C], f32, tag="rdb1_sb")
                nc.scalar.copy(rdb1_sb[:].rearrange("p t c -> p (t c)"), rdb1[:])
                nc.vector.tensor_mul(ot1s[0:64], ot1[0:64, :, 0:C], rdb1_sb[0:64])
                nc.vector.tensor_mul(ot1s[64:128], ot1[64:128, :, C:2 * C],
                                     rdb1_sb[64:128])

                # O-proj + residual -> x1 row layout (fp8-DR)
                wo8 = load_w8(wsa, wdr["sa_wo"])
                x1 = sp.tile([C, H], f32)
                for n in range(2):
                    acc = ps_mm.tile([128, 512], f32, tag="acc")
                    for kp in range(KP):
                        nc.tensor.matmul(acc[:C, :], ot1s[:, 2 * kp:2 * kp + 2, :],
                                         wo8[:, 2 * kp:2 * kp + 2,
                                             n * 512:(n + 1) * 512],
                                         start=(kp == 0), stop=(kp == KP - 1),
                                         perf_mode=DR)
                    nc.vector.tensor_add(x1[:, n * 512:(n + 1) * 512], acc[:C, :],
                                         bias_bcast["sa_bo"][:, n * 512:(n + 1) * 512])
                    nc.vector.tensor_add(x1[:, n * 512:(n + 1) * 512],
                                         x1[:, n * 512:(n + 1) * 512],
                                         x0[:, n * 512:(n + 1) * 512])

                y2 = ln_row(sp, x1, C, bias_bcast["ln2_g"], bias_bcast["ln2_b"], "ln2")
                y2t = sp.tile([128, KC, C], f8)
                transpose_chunks(sp, y2, [(y2t, None)])
                wq8c = load_w8(wsa, wdr["ca_wq"])
                qca = proj_dr(sp, y2t, wq8c, bias_bcast["ca_bq"], "q1")
                transpose_chunks(sp, qca, [(qt_eo, 0), (qt_eo, 1)])
                transpose_chunks(sp, x1, [(x1t, None)])

               # ============ PHASE 1: per-slot cross-attention ============
               if True:
                    def compute_v(g):
                        goff, nch = offs[g], nchs[g]
                        v = vp.tile([128, nchs[0], NH, DH], f8, tag="v",
                                    name=f"v_{g}")
                        for ch in range(nch):
                            coff = goff + ch * 128
                            for half in range(2):
                                acc = ps_mm.tile([128, 512], f32, tag="acc")
                                for kp in range(KP):
                                    nc.tensor.matmul(
                                        acc[:],
                                        xt[:, 2 * kp:2 * kp + 2, coff:coff + 128],
                                        wv_t[:, 2 * kp:2 * kp + 2,
                                             half * 512:(half + 1) * 512],
                                        start=(kp == 0), stop=(kp == KP - 1),
                                        perf_mode=DR)
                                nc.vector.tensor_add(
                                    v[:, ch, half * 8:(half + 1) * 8, :],
                                    acc[:].rearrange("p (h d) -> p h d", d=DH),
                                    bv_b[:, half * 512:(half + 1) * 512].rearrange(
                                        "p (h d) -> p h d", d=DH))
                        return v

                    cc0 = 0  # running chunk index into mb
                    for g in range(NG):
                        goff, nch = offs[g], nchs[g]
                        v = compute_v(g)

                        pt = ptp.tile([128, nchs[0], NH, C], f8, tag="pt")
                        for ch in range(nch):
                            coff = goff + ch * 128
                            for half in range(2):
                                st = ps_st.tile([128, 4, 2 * C], f32, tag="st")
                                for i in range(4):
                                    t = half * 4 + i
                                    nc.tensor.matmul(
                                        st[:, i, :],
                                        kt[:, t, coff:coff + 128],
                                        qt_eo[:, t, :],
                                        start=True, stop=True)
                                nc.scalar.activation(
                                    pt[:, ch, half * 8:(half + 1) * 8, :],
                                    st[:].rearrange("p a b -> p (a b)").rearrange(
                                        "p (h c) -> p h c", c=C),
                                    Act.Exp, bias=mb[:, cc0 + ch:cc0 + ch + 1],
                                    scale=SCALE)

                        dsb2 = [p1.tile([1, 512], f32r, tag=f"dsb{e}",
                                        name=f"dsb2_{e}") for e in range(2)]
                        nDR = nch // 2
                        for hf in range(2):
                            hs = slice(hf * 8, (hf + 1) * 8)
                            den = ps_den.tile([16, 512], f32, tag="den")
                            for cp2 in range(nDR):
                                nc.tensor.matmul(
                                    den[:], ones8p[:],
                                    pt[:, 2 * cp2:2 * cp2 + 2, hs, :].rearrange(
                                        "p a h c -> p a (h c)"),
                                    start=(cp2 == 0), stop=(nch % 2 == 0 and
                                                            cp2 == nDR - 1),
                                    perf_mode=DR)
                            if nch % 2:
                                nc.tensor.matmul(
                                    den[:], ones8w[:],
                                    pt[:, nch - 1, hs, :].rearrange(
                                        "p h c -> p (h c)"),
                                    start=(nDR == 0), stop=True)
                            with nc.allow_low_precision(
                                    reason="f32r rden for bcast matmul"):
                                nc.vector.reciprocal(dsb2[hf][:], den[0:1, :])
                        ot = ps_ot.tile([128, KC, 2 * C], f32, tag="ot")
                        for t in range(KC):
                            for cp2 in range(nDR):
                                nc.tensor.matmul(
                                    ot[:, t, :],
                                    v[:, 2 * cp2:2 * cp2 + 2, 2 * t:2 * t + 2,
                                      :].rearrange("p a b d -> p a (b d)"),
                                    pt[:, 2 * cp2:2 * cp2 + 2, 2 * t:2 * t + 2,
                                       :].rearrange("p a b c -> p a (b c)"),
                                    start=(cp2 == 0), stop=(nch % 2 == 0 and
                                                            cp2 == nDR - 1),
                                    perf_mode=DR)
                            if nch % 2:
                                nc.tensor.matmul(
                                    ot[:, t, :],
                                    v[:, nch - 1, 2 * t:2 * t + 2, :].rearrange(
                                        "p a d -> p (a d)"),
                                    pt[:, nch - 1, 2 * t:2 * t + 2, :].rearrange(
                                        "p a c -> p (a c)"),
                                    start=(nDR == 0), stop=True)
                        rdb = ps_rdb.tile([128, KC * C], f32, tag="rdb")
                        for hf in range(2):
                            dv = dsb2[hf][:].rearrange("p (h c) -> p h c", c=C)
                            for eo in range(2):
                                nc.tensor.matmul(
                                    rdb[:, hf * 256:(hf + 1) * 256],
                                    sel_eo[eo][:], dv[:, eo::2, :],
                                    start=(eo == 0), stop=(eo == 1))
                        rdb_sb = p1.tile([128, KC, C], f32, tag="rdb_sb")
                        nc.scalar.copy(rdb_sb[:].rearrange("p t c -> p (t c)"),
                                       rdb[:])
                        nc.vector.tensor_mul(ots[0:64, :, g, :],
                                             ot[0:64, :, 0:C], rdb_sb[0:64])
                        nc.vector.tensor_mul(ots[64:128, :, g, :],
                                             ot[64:128, :, C:2 * C],
                                             rdb_sb[64:128])
                        cc0 += nch

              # ============ PHASE 2a: O-projection (fp8-DR) ============
              with (tc.tile_pool(name="wop", bufs=1) as wop,
                    tc.tile_pool(name="sq0p", bufs=2) as sq0p,
                    tc.tile_pool(name="ps_st0", bufs=2, space="PSUM") as ps_st0):
                sum_ps0 = ps_st0.tile([1, NG * C], f32, tag="stat0")
                sq_ps0 = ps_st0.tile([1, NG * C], f32, tag="stat0")
                wo_t = load_w8(wop, wdr["ca_wo"], tag="woc")
                for m in range(MT):
                    acc = ps_mm.tile([128, 512], f32, tag="acc")
                    for kp in range(KP):
                        nc.tensor.matmul(
                            acc[:],
                            wo_t[:, 2 * kp:2 * kp + 2, m * 128:(m + 1) * 128],
                            ots[:, 2 * kp:2 * kp + 2, :, :].rearrange(
                                "p a g c -> p a (g c)"),
                            start=(kp == 0), stop=(kp == KP - 1),
                            perf_mode=DR)
                    nc.scalar.activation(x2t[:, m, :], acc[:], Act.Identity,
                                         bias=bo_t[:, m:m + 1])
                    nc.vector.tensor_add(
                        x2t[:, m, :].rearrange("p (g c) -> p g c", c=C),
                        x2t[:, m, :].rearrange("p (g c) -> p g c", c=C),
                        x1t[:, m, None, :].to_broadcast((128, NG, C)))
                    sq0 = sq0p.tile([128, NG * C], f32r, tag="sq0")
                    nc.vector.tensor_mul(sq0[:], x2t[:, m, :], x2t[:, m, :])
                    nc.tensor.matmul(sum_ps0[:], ones_r[:], x2t[:, m, :],
                                     start=(m == 0), stop=(m == MT - 1))
                    nc.tensor.matmul(sq_ps0[:], ones_r[:], sq0[:],
                                     start=(m == 0), stop=(m == MT - 1))
                nc.vector.tensor_copy(sum2_sb[:], sum_ps0[:])
                nc.vector.tensor_copy(sq2_sb[:], sq_ps0[:])

            # ============ PHASE 2: FFN (bf16), output ============
            with (
                tc.tile_pool(name="p2", bufs=1) as p2,
                tc.tile_pool(name="wstr", bufs=3) as wstr,
                tc.tile_pool(name="wstr2", bufs=2) as wstr2,
                tc.tile_pool(name="sq", bufs=2) as sqp,
                tc.tile_pool(name="ps_stat", bufs=4, space="PSUM") as ps_stat,
                tc.tile_pool(name="ps_bc", bufs=2, space="PSUM") as ps_bc,
            ):
                R = NG * C  # 512 rows
                # LN3 stats were accumulated during O-proj (sum2_sb/sq2_sb)
                mean = p2.tile([1, R], f32r, tag="mean")
                with nc.allow_low_precision(reason="f32r mean/rstd for K=1 bcast matmul"):
                    nc.scalar.mul(mean[:], sum2_sb[:], 1.0 / H)
                var = p2.tile([1, R], f32, tag="var")
                nc.scalar.mul(var[:], sq2_sb[:], 1.0 / H)
                m2 = p2.tile([1, R], f32, tag="m2")
                nc.vector.tensor_mul(m2[:], mean[:], mean[:])
                nc.vector.tensor_sub(var[:], var[:], m2[:])
                rstd = p2.tile([1, R], f32r, tag="rstd")
                with nc.allow_low_precision(reason="f32r mean/rstd for K=1 bcast matmul"):
                    nc.scalar.activation(rstd[:], var[:], Act.Sqrt, bias=eps_t[0:1])
                    nc.vector.reciprocal(rstd[:], rstd[:])
                mean_b = ps_bc.tile([128, R], f32, tag="bc")
                rstd_b = ps_bc.tile([128, R], f32, tag="bc")
                nc.tensor.matmul(mean_b[:], onesrow_r[:], mean[:],
                                 start=True, stop=True)
                nc.tensor.matmul(rstd_b[:], onesrow_r[:], rstd[:],
                                 start=True, stop=True)

                y3t = p2.tile([128, KC, R], bf16, tag="y3t")
                for m in range(MT):
                    nc.vector.tensor_sub(y3t[:, m, :], x2t[:, m, :], mean_b[:])
                    nc.vector.tensor_mul(y3t[:, m, :], y3t[:, m, :], rstd_b[:])
                    nc.vector.tensor_scalar(
                        y3t[:, m, :], y3t[:, m, :],
                        scalar1=g3_t[:, m:m + 1], scalar2=b3_t[:, m:m + 1],
                        op0=mybir.AluOpType.mult, op1=mybir.AluOpType.add)

                # GEMM1: h1T [128, FM, R] bf16
                h1t = p2.tile([128, FM, R], bf16, tag="h1t")
                w1_re = w1_d.ap().rearrange("(k p) f -> p k f", p=128)
                for fm in range(FM):
                    w1c = wstr.tile([128, KC, 128], bf16, tag="w1c")
                    for k4 in range(0, KC, 4):
                        nc.sync.dma_start(w1c[:, k4:k4 + 4, :],
                                          w1_re[:, k4:k4 + 4, fm * 128:(fm + 1) * 128])
                    acc = ps_mm.tile([128, 512], f32, tag="acc")
                    for k in range(KC):
                        nc.tensor.matmul(acc[:], w1c[:, k, :], y3t[:, k, :],
                                         start=(k == 0), stop=(k == KC - 1))
                    nc.scalar.activation(h1t[:, fm, :], acc[:], Act.Relu,
                                         bias=b1_t[:, fm:fm + 1])

                # GEMM2: x3T = W2^T-chunks @ h1T + x2T + b2
                sum3 = ps_stat.tile([1, R], f32, tag="stat")
                sq3 = ps_stat.tile([1, R], f32, tag="stat")
                a_ps = ps_stat.tile([1, R], f32, tag="stat")
                x3t = p2.tile([128, MT, R], f32r, tag="x3t")
                w2_re = w2_d.ap().rearrange("(k p) f -> p k f", p=128)
                w2cs = {}

                def load_w2c(m):
                    w2c = wstr2.tile([128, FM, 128], bf16, tag="w2c")
                    for f8_ in range(0, FM, 8):
                        nc.gpsimd.dma_start(
                            w2c[:, f8_:f8_ + 8, :],
                            w2_re[:, f8_:f8_ + 8, m * 128:(m + 1) * 128])
                    w2cs[m] = w2c
                load_w2c(0)
                for m in range(MT):
                    w2c = w2cs.pop(m)
                    if m + 1 < MT:
                        load_w2c(m + 1)
                    acc = ps_mm.tile([128, 512], f32, tag="acc")
                    for fk in range(FM):
                        nc.tensor.matmul(acc[:], w2c[:, fk, :], h1t[:, fk, :],
                                         start=(fk == 0), stop=(fk == FM - 1))
                    nc.scalar.activation(x3t[:, m, :], acc[:], Act.Identity,
                                         bias=b2_t[:, m:m + 1])
                    nc.vector.tensor_add(x3t[:, m, :], x3t[:, m, :],
                                         x2t[:, m, :])
                    sq = sqp.tile([128, R], f32r, tag="sq")
                    nc.vector.tensor_mul(sq[:], x3t[:, m, :], x3t[:, m, :])
                    nc.tensor.matmul(sum3[:], ones_r[:], x3t[:, m, :],
                                     start=(m == 0), stop=(m == MT - 1))
                    nc.tensor.matmul(sq3[:], ones_r[:], sq[:],
                                     start=(m == 0), stop=(m == MT - 1))
                    nc.tensor.matmul(a_ps[:], wg_t[:, m:m + 1], x3t[:, m, :],
                                     start=(m == 0), stop=(m == MT - 1))

                # final LN + linear folded: logits = rstd*(A - mean*S1) + S2
                mean3 = p2.tile([1, R], f32, tag="mean3")
                nc.scalar.mul(mean3[:], sum3[:], 1.0 / H)
                var3 = p2.tile([1, R], f32, tag="var3")
                nc.scalar.mul(var3[:], sq3[:], 1.0 / H)
                m23 = p2.tile([1, R], f32, tag="m23")
                nc.vector.tensor_mul(m23[:], mean3[:], mean3[:])
                nc.vector.tensor_sub(var3[:], var3[:], m23[:])
                rstd3 = p2.tile([1, R], f32, tag="rstd3")
                nc.scalar.activation(rstd3[:], var3[:], Act.Sqrt, bias=eps_t[0:1])
                nc.vector.reciprocal(rstd3[:], rstd3[:])
                logits = p2.tile([1, R], f32, tag="logits")
                nc.scalar.mul(logits[:], mean3[:], -S1)
                nc.vector.tensor_add(logits[:], logits[:], a_ps[:])
                nc.vector.tensor_mul(logits[:], logits[:], rstd3[:])
                nc.scalar.add(logits[:], logits[:], S2)
                nc.sync.dma_start(out_d.ap()[None, :], logits[:])

    nc.compile()
    return nc


def _ceil16(n):
    return -(-int(n) // 16) * 16


def _prep(inputs):
    nf = np.ascontiguousarray(np.asarray(inputs["node_features"], np.float32))
    batch = np.asarray(inputs["batch"]).astype(np.int64)
    counts = np.bincount(batch, minlength=B)
    offsets = np.concatenate([[0], np.cumsum(counts)[:-1]])

    # slot packing: rank graphs by count desc; slot j holds ranks [8j, 8j+8)
    order = np.argsort(-counts, kind="stable")
    L = [_ceil16(max(int(counts[order[j * NCORES]]), 16)) for j in range(NG)]
    offs = np.concatenate([[0], np.cumsum(L)[:-1]]).astype(int).tolist()
    nchs = [-(-l // 128) for l in L]
    TOT_ALLOC = _ceil16(offs[-1] + nchs[-1] * 128)
    meta = (offs, nchs, TOT_ALLOC)

    xts, mbs = [], []
    NCHSUM = sum(nchs)
    for c in range(NCORES):
        xt = np.zeros((H, TOT_ALLOC), np.float32)
        mbm = np.full((NCHSUM, 128), NEG, np.float32)
        cc0 = 0
        for j in range(NG):
            g = int(order[j * NCORES + c])
            n = int(counts[g])
            o = int(offsets[g])
            xt[:, offs[j]:offs[j] + n] = nf[o:o + n].T
            # empty graph guard: unmask one zero-feature key
            nv = max(n, 1)
            for ch in range(nchs[j]):
                lo = ch * 128
                mbm[cc0 + ch, :max(0, min(nv - lo, 128))] = 0.0
            cc0 += nchs[j]
        xts.append(np.clip(xt, -240.0, 240.0).astype(F8))
        mbs.append(np.ascontiguousarray(mbm.T.reshape(-1)))

    out_w = np.asarray(inputs["out_w"], np.float32)[:, 0]
    og = np.asarray(inputs["out_ln_g"], np.float32)
    ob = np.asarray(inputs["out_ln_b"], np.float32)
    wg = (out_w * og).astype(np.float32)
    S1 = float(wg.sum())
    S2 = float((out_w * ob).sum() + np.asarray(inputs["out_b"], np.float32)[0])

    def perm128(v):
        return np.asarray(v, np.float32).reshape(-1, 128).T

    pbias = np.zeros((128, 80), np.float32)
    pbias[:, 0:8] = perm128(inputs["ca_bk"])
    pbias[:, 8:16] = perm128(inputs["ca_bo"])
    pbias[:, 16:24] = perm128(inputs["ln3_g"])
    pbias[:, 24:32] = perm128(inputs["ln3_b"])
    pbias[:, 32:40] = perm128(inputs["ff_b2"])
    pbias[:, 40:48] = perm128(wg)
    pbias[:, 48:80] = perm128(inputs["ff_b1"])
    sel2 = np.zeros((2, 128), np.float32)
    sel2[0, 0:64] = 1.0
    sel2[1, 64:128] = 1.0
    common = {"cq": np.ascontiguousarray(np.asarray(inputs["class_queries"], np.float32)),
              "wg": wg, "sel2": sel2, "pbias": pbias}
    for nm in ("sa_wq", "sa_wk", "sa_wv", "sa_wo", "ca_wq", "ca_wk", "ca_wv",
               "ca_wo"):
        common[nm] = np.clip(np.asarray(inputs[nm], np.float32),
                             -240.0, 240.0).astype(F8)
    for nm in ("ff_w1", "ff_w2"):
        common[nm] = np.asarray(inputs[nm], np.float32).astype(BF16)
    for nm in ("sa_bq", "sa_bk", "sa_bv", "sa_bo", "ca_bq", "ca_bk",
               "ca_bv", "ca_bo", "ln1_g", "ln1_b", "ln2_g", "ln2_b",
               "ln3_g", "ln3_b", "ff_b1", "ff_b2"):
        common[nm] = np.ascontiguousarray(np.asarray(inputs[nm], np.float32))
    common["bias10"] = np.stack(
        [np.asarray(inputs[nm], np.float32) for nm in
         ("sa_bq", "sa_bk", "sa_bv", "sa_bo", "ca_bq",
          "ln1_g", "ln1_b", "ln2_g", "ln2_b", "ca_bv")]).astype(BF16)

    in_maps = []
    for c in range(NCORES):
        m = dict(common)
        m["xt"] = xts[c]
        m["mb"] = mbs[c]
        in_maps.append(m)
    return meta, S1, S2, in_maps, order


def _run(inputs, trace=False):
    from concourse.bass_utils import run_bass_kernel_spmd
    meta, S1, S2, in_maps, order = _prep(inputs)
    nc = build_nc(meta, S1, S2)
    try:
        r = run_bass_kernel_spmd(nc, in_maps, core_ids=list(range(NCORES)),
                                 trace=trace)
    except Exception:
        # transient device wedge (NRT_EXEC_UNIT_UNRECOVERABLE) clears on retry
        r = run_bass_kernel_spmd(nc, in_maps, core_ids=list(range(NCORES)),
                                 trace=trace)
    out = np.zeros((B, C), np.float32)
    for c in range(NCORES):
        rc = r.results[c]["out"].reshape(NG, C)
        for j in range(NG):
            out[int(order[j * NCORES + c])] = rc[j]
    return out.astype(np.float32), r


def kernel(**inputs):
    return _run(inputs, trace=False)[0]


# revision 37
# speedup vs baseline: 1.0646x; 1.0295x over previous
"""Trainium2 Bass kernel for nn_ClassQueryHead (transformer decoder head over
ragged graph batches).

Strategy: data-parallel over graphs (8 graphs per core x 8 cores), with
size-balanced slot packing: graphs sorted by node count, slot j on every core
holds one of ranks [8j, 8j+8), padded to L_j = ceil16(max count in slot) --
~4080 key columns per core instead of 8*640.

Device pipeline per core:
  stage A (shared): self-attn block on class queries, fp8-DoubleRow
    projections (weights+activations e4m3, fp32 accumulate)
  phase 1: K projection for the whole core's packed columns (fp8-DR),
    then per slot: V projection (fp8-DR), scores (fp8 matmul vs shared
    query tile), exp fused on ACT with the ragged mask as per-partition
    bias, denominator via fp8-DR ones-matmul, PV (fp8 matmul)
  phase 2: O-projection (fp8-DR) into the transposed residual stream,
    LayerNorm via ones-matmul partition reductions, FFN in bf16 (fp8
    breaks the 2e-2 error budget), final LN+Linear folded into one
    matmul with wg = out_w * ln_g.
"""
import numpy as np
import ml_dtypes

H = 1024
NH = 16
DH = 64
C = 64
B = 64
FF = 4096
EPS = 1e-5
SCALE = 0.125
NCORES = 8
NG = B // NCORES  # graphs (slots) per core
KC = H // 128     # contract chunks of H
KP = KC // 2      # fp8 DoubleRow pairs
MT = H // 128     # m tiles of H
FM = FF // 128    # ff tiles
NEG = -1e30
F8 = ml_dtypes.float8_e4m3
BF16 = ml_dtypes.bfloat16


def _pieces(n):
    """Split free dim n into pieces <=512."""
    out = []
    rem = n
    while rem > 512:
        out.append(512)
        rem -= 512
    if rem:
        out.append(rem)
    return out


def build_nc(meta, S1, S2):
    import concourse.bass as bass
    import concourse.tile as tile
    import concourse.mybir as mybir
    from concourse import bacc
    from concourse.masks import make_identity

    offs, nchs, TOT_ALLOC = meta
    NCHSUM = sum(nchs)
    f32 = mybir.dt.float32
    f32r = mybir.dt.float32r
    bf16 = mybir.dt.bfloat16
    f8 = mybir.dt.float8e4
    Act = mybir.ActivationFunctionType
    DR = mybir.MatmulPerfMode.DoubleRow

    nc = bacc.Bacc("TRN2", target_bir_lowering=False, debug=False,
                   num_devices=NCORES)

    # ---- DRAM I/O ----
    xt_d = nc.dram_tensor("xt", [H, TOT_ALLOC], f8, kind="ExternalInput")
    mb_d = nc.dram_tensor("mb", [NCHSUM * 128], f32, kind="ExternalInput")
    cq_d = nc.dram_tensor("cq", [C, H], f32, kind="ExternalInput")
    wdr = {}
    for nm in ("sa_wq", "sa_wk", "sa_wv", "sa_wo", "ca_wq", "ca_wk", "ca_wv",
               "ca_wo"):
        wdr[nm] = nc.dram_tensor(nm, [H, H], f8, kind="ExternalInput")
    bdr = {}
    for nm in ("sa_bq", "sa_bk", "sa_bv", "sa_bo", "ca_bq", "ca_bk", "ca_bv",
               "ca_bo", "ln1_g", "ln1_b", "ln2_g", "ln2_b", "ln3_g", "ln3_b",
               "ff_b2"):
        bdr[nm] = nc.dram_tensor(nm, [H], f32, kind="ExternalInput")
    bias10_d = nc.dram_tensor("bias10", [10, H], bf16, kind="ExternalInput")
    pbias_d = nc.dram_tensor("pbias", [128, 80], f32, kind="ExternalInput")
    w1_d = nc.dram_tensor("ff_w1", [H, FF], bf16, kind="ExternalInput")
    b1_d = nc.dram_tensor("ff_b1", [FF], f32, kind="ExternalInput")
    w2_d = nc.dram_tensor("ff_w2", [FF, H], bf16, kind="ExternalInput")
    wg_d = nc.dram_tensor("wg", [H], f32r, kind="ExternalInput")
    sel2_d = nc.dram_tensor("sel2", [2, 128], f32r, kind="ExternalInput")
    out_d = nc.dram_tensor("out", [NG * C], f32, kind="ExternalOutput")

    def bcast_load(nc, out_ap, dram, nparts, offset=0, inner=H):
        src = bass.AP(tensor=dram.ap().tensor, offset=offset,
                      ap=[[0, nparts], [1, inner]])
        nc.gpsimd.dma_start(out=out_ap, in_=src)

    with tile.TileContext(nc) as tc:
        with (
            tc.tile_pool(name="const", bufs=1) as cp,
            tc.tile_pool(name="ps_mm", bufs=2, space="PSUM") as ps_mm,
            tc.tile_pool(name="dram", bufs=2, space="DRAM") as drp,
        ):
            ident = cp.tile([128, 128], f32)
            make_identity(nc, ident[:])
            ones_f = cp.tile([128, 1], f32)
            nc.vector.memset(ones_f[:], 1.0)
            ones_r = cp.tile([128, 1], f32r)
            nc.vector.tensor_copy(ones_r[:], ones_f[:])
            # dual-fp8 LDWEIGHTS needs >=16 cols per k-plane (walrus
            # s3_lw_dual_fp8_restrictions), so the DR ones is 16 wide and
            # only partition 0 of its output is read.
            ones8p = cp.tile([128, 2, 16], f8)
            nc.vector.memset(ones8p[:], 1.0)
            ones8w = cp.tile([128, 16], f8)
            nc.vector.memset(ones8w[:], 1.0)
            ones_b = cp.tile([128, 16], bf16)
            nc.vector.memset(ones_b[:], 1.0)
            sel_eo = []
            for eo in range(2):
                t = cp.tile([1, 128], f32r, tag=f"sel{eo}")
                nc.scalar.dma_start(t[:], sel2_d.ap()[eo, None, :])
                sel_eo.append(t)
            onesrow_f = cp.tile([1, 128], f32)
            nc.vector.memset(onesrow_f[:], 1.0)
            onesrow_r = cp.tile([1, 128], f32r)
            nc.vector.tensor_copy(onesrow_r[:], onesrow_f[:])
            eps_t = cp.tile([128, 1], f32)
            nc.vector.memset(eps_t[:], EPS)

            # per-partition bias tiles, host-packed contiguous [128, 80]
            pb = cp.tile([128, 80], f32)
            nc.scalar.dma_start(pb[:], pbias_d.ap())
            bk_t = pb[:, 0:8]
            bo_t = pb[:, 8:16]
            g3_t = pb[:, 16:24]
            b3_t = pb[:, 24:32]
            b2_t = pb[:, 32:40]
            b1_t = pb[:, 48:80]
            wg_t = cp.tile([128, MT], f32r)
            nc.vector.tensor_copy(wg_t[:], pb[:, 40:48])
            # stage-A-critical loads first on the scalar queue (the ring
            # serializes transfers; these gate the ln1 chain during kt)
            bias_bcast = {}
            for bi, nm in enumerate(("sa_bq", "sa_bk", "sa_bv", "sa_bo",
                                     "ca_bq", "ln1_g", "ln1_b", "ln2_g",
                                     "ln2_b")):
                t = cp.tile([C, H], bf16, tag=f"bb_{nm}")
                bsrc = bass.AP(tensor=bias10_d.ap().tensor, offset=bi * H,
                               ap=[[0, C], [1, H]])
                nc.scalar.dma_start(out=t[:], in_=bsrc)
                bias_bcast[nm] = t
            x0 = cp.tile([C, H], f32)
            nc.scalar.dma_start(x0[:], cq_d.ap())
            bv_b = cp.tile([128, H], bf16)
            nc.scalar.dma_start(out=bv_b[:], in_=bass.AP(
                tensor=bias10_d.ap().tensor, offset=9 * H,
                ap=[[0, 128], [1, H]]))

            # persistent activations
            x1t = cp.tile([128, KC, C], f32)        # x1 transposed
            qt_eo = cp.tile([128, KC, 2 * C], f8)   # [q_even | q_odd], zero-pad
            nc.vector.memset(qt_eo[:], 0.0)
            x2t = cp.tile([128, MT, NG * C], f32r)  # residual stream T
            sum2_sb = cp.tile([1, NG * C], f32)
            sq2_sb = cp.tile([1, NG * C], f32)

            def ln_row(pool, x, n_p, g_b, b_b, name):
                """LayerNorm on row-layout x [n_p, H] -> new tile."""
                stats = pool.tile([n_p, 2, 6], f32, tag="ln_st")
                for i in range(2):
                    nc.vector.bn_stats(stats[:, i, :], x[:, i * 512:(i + 1) * 512])
                mv = pool.tile([n_p, 2], f32, tag="ln_mv")
                nc.vector.bn_aggr(mv[:], stats[:])
                rstd = pool.tile([n_p, 1], f32, tag="ln_rs")
                nc.scalar.activation(rstd[:], mv[:, 1:2], Act.Sqrt,
                                     bias=eps_t[:n_p])
                nc.vector.reciprocal(rstd[:], rstd[:])
                y = pool.tile([n_p, H], f32, tag="ln_y")
                nc.vector.tensor_scalar(y[:], x[:], scalar1=mv[:, 0:1],
                                        scalar2=rstd[:],
                                        op0=mybir.AluOpType.subtract,
                                        op1=mybir.AluOpType.mult)
                nc.vector.tensor_mul(y[:], y[:], g_b[:])
                nc.vector.tensor_add(y[:], y[:], b_b[:])
                return y

            def transpose_chunks(pool, src, dst_list, n_p=C):
                """PE-transpose src [n_p, H] into dst slices [128, k, n_p]."""
                for k in range(KC):
                    tp = ps_mm.tile([128, 512], f32, tag="acc")
                    nc.tensor.transpose(tp[:, :n_p], src[:, k * 128:(k + 1) * 128],
                                        ident[:n_p, :n_p])
                    for dst, par in dst_list:
                        if par is None:
                            nc.scalar.copy(dst[:, k, :], tp[:, :n_p])
                        elif par == 0:
                            nc.scalar.copy(dst[0:64, k, 0:n_p], tp[0:64, :n_p])
                        else:
                            nc.scalar.copy(dst[64:128, k, n_p:2 * n_p],
                                           tp[64:128, :n_p])

            def load_w8(pool, w_dram, tag="w8", engs=None):
                """Load [H, H] fp8 weight as [128, KC, H]."""
                w_re = w_dram.ap().rearrange("(k p) n -> p k n", p=128)
                w_t = pool.tile([128, KC, H], f8, tag=tag)
                engs = engs or (nc.sync, nc.gpsimd)
                for i, k2 in enumerate(range(0, KC, 2)):
                    eng = engs[i % len(engs)]
                    eng.dma_start(w_t[:, k2:k2 + 2, :], w_re[:, k2:k2 + 2, :])
                return w_t

            def proj_dr(pool, yt8, w8, bias_b, name, out_dt=f32):
                """fp8-DR projection: out [C, H] = y @ W + b (row layout)."""
                o = pool.tile([C, H], out_dt, tag=f"{name}_o")
                for n in range(2):
                    acc = ps_mm.tile([128, 512], f32, tag="acc")
                    for kp in range(KP):
                        nc.tensor.matmul(acc[:C, :], yt8[:, 2 * kp:2 * kp + 2, :],
                                         w8[:, 2 * kp:2 * kp + 2,
                                            n * 512:(n + 1) * 512],
                                         start=(kp == 0), stop=(kp == KP - 1),
                                         perf_mode=DR)
                    nc.vector.tensor_add(o[:, n * 512:(n + 1) * 512],
                                         acc[:C, :], bias_b[:, n * 512:(n + 1) * 512])
                return o

            # ============ STAGE A + PHASE 1 ============
            with tc.tile_pool(name="mid", bufs=1) as midp:
              ots = midp.tile([128, KC, NG, C], f8)  # attn out T, all slots
              with (
                tc.tile_pool(name="ps_st", bufs=2, space="PSUM") as ps_st,
                tc.tile_pool(name="ps_ot", bufs=1, space="PSUM") as ps_ot,
                tc.tile_pool(name="ps_den", bufs=1, space="PSUM") as ps_den,
                tc.tile_pool(name="ps_rdb", bufs=1, space="PSUM") as ps_rdb,
                tc.tile_pool(name="p1", bufs=1) as p1,
                tc.tile_pool(name="vp", bufs=2) as vp,
                tc.tile_pool(name="ptp", bufs=2) as ptp,
              ):
               # phase-1 inputs stream first so the K-projection can start
               # as soon as the tensor queue drains
               mb = p1.tile([128, NCHSUM], f32, tag="mb")
               nc.scalar.dma_start(mb[:], mb_d.ap().rearrange("(p c) -> p c", p=128))
               wkp = tc.alloc_tile_pool(name="wkp", bufs=1)
               wk_t = load_w8(wkp, wdr["ca_wk"], tag="wkc", engs=(nc.sync,))
               xt = p1.tile([128, KC, TOT_ALLOC], f8, tag="xt")
               xt_re = xt_d.ap().rearrange("(k p) n -> p k n", p=128)
               # column-blocked load so the K-projection can start after the
               # first ~1MB block instead of the whole 4MB tensor
               blocks = []
               boff = 0
               while boff < TOT_ALLOC:
                   bw = min(1024, TOT_ALLOC - boff)
                   blocks.append((boff, bw))
                   boff += bw
               for boff, bw in blocks:
                   for k2 in range(0, KC, 2):
                       eng = nc.sync if k2 % 4 == 0 else nc.gpsimd
                       eng.dma_start(xt[:, k2:k2 + 2, boff:boff + bw],
                                     xt_re[:, k2:k2 + 2, boff:boff + bw])
               wv_t = load_w8(p1, wdr["ca_wv"], tag="wvc", engs=(nc.gpsimd,))

               # K-projection for the whole packed column space (dense DR
               # stream, no deps on stage A); wk frees before stage A opens
               kt = p1.tile([128, MT, TOT_ALLOC], f8, tag="kt")
               if True:
                   for boff, bw in blocks:
                       for m in range(MT):
                           off = boff
                           for pc in _pieces(bw):
                               acc = ps_mm.tile([128, 512], f32, tag="acc")
                               for kp in range(KP):
                                   nc.tensor.matmul(
                                       acc[:, :pc],
                                       wk_t[:, 2 * kp:2 * kp + 2,
                                            m * 128:(m + 1) * 128],
                                       xt[:, 2 * kp:2 * kp + 2, off:off + pc],
                                       start=(kp == 0), stop=(kp == KP - 1),
                                       perf_mode=DR)
                               nc.scalar.activation(
                                   kt[:, m, off:off + pc], acc[:, :pc],
                                   Act.Identity, bias=bk_t[:, m:m + 1])
                               off += pc

               wkp.release()
               with (tc.tile_pool(name="sa", bufs=1) as sp,
                     tc.tile_pool(name="wsa", bufs=2) as wsa):
                wq8 = load_w8(wsa, wdr["sa_wq"])
                wk8 = load_w8(wsa, wdr["sa_wk"])
                wv8 = load_w8(wsa, wdr["sa_wv"])
                y1 = ln_row(sp, x0, C, bias_bcast["ln1_g"], bias_bcast["ln1_b"], "ln1")
                y1t = sp.tile([128, KC, C], f8)
                transpose_chunks(sp, y1, [(y1t, None)])
                q1 = proj_dr(sp, y1t, wq8, bias_bcast["sa_bq"], "q1")
                k1 = proj_dr(sp, y1t, wk8, bias_bcast["sa_bk"], "k1")
                v1 = proj_dr(sp, y1t, wv8, bias_bcast["sa_bv"], "v1")

                k1t = sp.tile([128, KC, C], bf16)
                transpose_chunks(sp, k1, [(k1t, None)])
                q1t_eo = sp.tile([128, KC, 2 * C], bf16)
                nc.vector.memset(q1t_eo[:], 0.0)
                transpose_chunks(sp, q1, [(q1t_eo, 0), (q1t_eo, 1)])
                v1b = sp.tile([128, NH, DH], bf16)
                nc.vector.memset(v1b[:], 0.0)
                nc.vector.tensor_copy(
                    v1b[0:64, :, :], v1[:].rearrange("p (h d) -> p h d", d=DH))

                # self-attn scores/exp (keys=64, one chunk)
                pt1 = sp.tile([128, NH, C], bf16)
                nc.vector.memset(pt1[:], 0.0)
                for half in range(2):
                    st = ps_st.tile([128, 4, 2 * C], f32, tag="st")
                    for i in range(4):
                        t = half * 4 + i
                        nc.tensor.matmul(st[:C, i, :], k1t[:, t, :],
                                         q1t_eo[:, t, :], start=True, stop=True)
                    nc.scalar.activation(
                        pt1[0:C, half * 8:(half + 1) * 8, :],
                        st[:C, :, :].rearrange("p a b -> p (a b)").rearrange(
                            "p (h c) -> p h c", c=C),
                        Act.Exp, bias=0.0, scale=SCALE)
                dsb = [sp.tile([1, 512], f32r, tag=f"dsb{e}", name=f"dsb_a{e}")
                       for e in range(2)]
                for hf in range(2):
                    den1 = ps_den.tile([16, 512], f32, tag="den")
                    nc.tensor.matmul(
                        den1[:], ones_b[:],
                        pt1[:, hf * 8:(hf + 1) * 8, :].rearrange(
                            "p h c -> p (h c)"),
                        start=True, stop=True)
                    with nc.allow_low_precision(reason="f32r rden for bcast matmul"):
                        nc.vector.reciprocal(dsb[hf][:], den1[0:1, :])
                ot1 = ps_ot.tile([128, KC, 2 * C], f32, tag="ot")
                for t in range(KC):
                    nc.tensor.matmul(
                        ot1[:, t, :],
                        v1b[:, 2 * t:2 * t + 2, :].rearrange("p a d -> p (a d)"),
                        pt1[:, 2 * t:2 * t + 2, :].rearrange("p a c -> p (a c)"),
                        start=True, stop=True)
                # broadcast 1/den across partitions: accumulate
                # sel_e⊗even + sel_o⊗odd heads (full-128 dst, ISA-safe);
                # dsb half hf holds heads 8hf..8hf+7 -> strided eo slice
                rdb1 = ps_rdb.tile([128, KC * C], f32, tag="rdb")
                for hf in range(2):
                    dv = dsb[hf][:].rearrange("p (h c) -> p h c", c=C)
                    for eo in range(2):
                        nc.tensor.matmul(rdb1[:, hf * 256:(hf + 1) * 256],
                                         sel_eo[eo][:], dv[:, eo::2, :],
                                         start=(eo == 0), stop=(eo == 1))
                ot1s = sp.tile([128, KC, C], f8)
                rdb1_sb = sp.tile([128, KC, C], f32, tag="rdb1_sb")
                nc.scalar.copy(rdb1_sb[:].rearrange("p t c -> p (t c)"), rdb1[:])
                nc.vector.tensor_mul(ot1s[0:64], ot1[0:64, :, 0:C], rdb1_sb[0:64])
                nc.vector.tensor_mul(ot1s[64:128], ot1[64:128, :, C:2 * C],
                                     rdb1_sb[64:128])

                # O-proj + residual -> x1 row layout (fp8-DR)
                wo8 = load_w8(wsa, wdr["sa_wo"])
                x1 = sp.tile([C, H], f32)
                for n in range(2):
                    acc = ps_mm.tile([128, 512], f32, tag="acc")
                    for kp in range(KP):
                        nc.tensor.matmul(acc[:C, :], ot1s[:, 2 * kp:2 * kp + 2, :],
                                         wo8[:, 2 * kp:2 * kp + 2,
                                             n * 512:(n + 1) * 512],
                                         start=(kp == 0), stop=(kp == KP - 1),
                                         perf_mode=DR)
                    nc.vector.tensor_add(x1[:, n * 512:(n + 1) * 512], acc[:C, :],
                                         bias_bcast["sa_bo"][:, n * 512:(n + 1) * 512])
                    nc.vector.tensor_add(x1[:, n * 512:(n + 1) * 512],
                                         x1[:, n * 512:(n + 1) * 512],
                                         x0[:, n * 512:(n + 1) * 512])

                y2 = ln_row(sp, x1, C, bias_bcast["ln2_g"], bias_bcast["ln2_b"], "ln2")
                y2t = sp.tile([128, KC, C], f8)
                transpose_chunks(sp, y2, [(y2t, None)])
                wq8c = load_w8(wsa, wdr["ca_wq"])
                qca = proj_dr(sp, y2t, wq8c, bias_bcast["ca_bq"], "q1")
                transpose_chunks(sp, qca, [(qt_eo, 0), (qt_eo, 1)])
                transpose_chunks(sp, x1, [(x1t, None)])

               # ============ PHASE 1: per-slot cross-attention ============
               if True:
                    def compute_v(g):
                        goff, nch = offs[g], nchs[g]
                        v = vp.tile([128, nchs[0], NH, DH], f8, tag="v",
                                    name=f"v_{g}")
                        for ch in range(nch):
                            coff = goff + ch * 128
                            for half in range(2):
                                acc = ps_mm.tile([128, 512], f32, tag="acc")
                                for kp in range(KP):
                                    nc.tensor.matmul(
                                        acc[:],
                                        xt[:, 2 * kp:2 * kp + 2, coff:coff + 128],
                                        wv_t[:, 2 * kp:2 * kp + 2,
                                             half * 512:(half + 1) * 512],
                                        start=(kp == 0), stop=(kp == KP - 1),
                                        perf_mode=DR)
                                nc.vector.tensor_add(
                                    v[:, ch, half * 8:(half + 1) * 8, :],
                                    acc[:].rearrange("p (h d) -> p h d", d=DH),
                                    bv_b[:, half * 512:(half + 1) * 512].rearrange(
                                        "p (h d) -> p h d", d=DH))
                        return v

                    cc0 = 0  # running chunk index into mb
                    for g in range(NG):
                        goff, nch = offs[g], nchs[g]
                        v = compute_v(g)

                        pt = ptp.tile([128, nchs[0], NH, C], f8, tag="pt")
                        for ch in range(nch):
                            coff = goff + ch * 128
                            for half in range(2):
                                st = ps_st.tile([128, 4, 2 * C], f32, tag="st")
                                for i in range(4):
                                    t = half * 4 + i
                                    nc.tensor.matmul(
                                        st[:, i, :],
                                        kt[:, t, coff:coff + 128],
                                        qt_eo[:, t, :],
                                        start=True, stop=True)
                                nc.scalar.activation(
                                    pt[:, ch, half * 8:(half + 1) * 8, :],
                                    st[:].rearrange("p a b -> p (a b)").rearrange(
                                        "p (h c) -> p h c", c=C),
                                    Act.Exp, bias=mb[:, cc0 + ch:cc0 + ch + 1],
                                    scale=SCALE)

                        dsb2 = [p1.tile([1, 512], f32r, tag=f"dsb{e}",
                                        name=f"dsb2_{e}") for e in range(2)]
                        nDR = nch // 2
                        for hf in range(2):
                            hs = slice(hf * 8, (hf + 1) * 8)
                            den = ps_den.tile([16, 512], f32, tag="den")
                            for cp2 in range(nDR):
                                nc.tensor.matmul(
                                    den[:], ones8p[:],
                                    pt[:, 2 * cp2:2 * cp2 + 2, hs, :].rearrange(
                                        "p a h c -> p a (h c)"),
                                    start=(cp2 == 0), stop=(nch % 2 == 0 and
                                                            cp2 == nDR - 1),
                                    perf_mode=DR)
                            if nch % 2:
                                nc.tensor.matmul(
                                    den[:], ones8w[:],
                                    pt[:, nch - 1, hs, :].rearrange(
                                        "p h c -> p (h c)"),
                                    start=(nDR == 0), stop=True)
                            with nc.allow_low_precision(
                                    reason="f32r rden for bcast matmul"):
                                nc.vector.reciprocal(dsb2[hf][:], den[0:1, :])
                        ot = ps_ot.tile([128, KC, 2 * C], f32, tag="ot")
                        for t in range(KC):
                            for cp2 in range(nDR):
                                nc.tensor.matmul(
                                    ot[:, t, :],
                                    v[:, 2 * cp2:2 * cp2 + 2, 2 * t:2 * t + 2,
                                      :].rearrange("p a b d -> p a (b d)"),
                                    pt[:, 2 * cp2:2 * cp2 + 2, 2 * t:2 * t + 2,
                                       :].rearrange("p a b c -> p a (b c)"),
                                    start=(cp2 == 0), stop=(nch % 2 == 0 and
                                                            cp2 == nDR - 1),
                                    perf_mode=DR)
                            if nch % 2:
                                nc.tensor.matmul(
                                    ot[:, t, :],
                                    v[:, nch - 1, 2 * t:2 * t + 2, :].rearrange(
                                        "p a d -> p (a d)"),
                                    pt[:, nch - 1, 2 * t:2 * t + 2, :].rearrange(
                                        "p a c -> p (a c)"),
                                    start=(nDR == 0), stop=True)
                        rdb = ps_rdb.tile([128, KC * C], f32, tag="rdb")
                        for hf in range(2):
                            dv = dsb2[hf][:].rearrange("p (h c) -> p h c", c=C)
                            for eo in range(2):
                                nc.tensor.matmul(
                                    rdb[:, hf * 256:(hf + 1) * 256],
                                    sel_eo[eo][:], dv[:, eo::2, :],
                                    start=(eo == 0), stop=(eo == 1))
                        rdb_sb = p1.tile([128, KC, C], f32, tag="rdb_sb")
                        nc.scalar.copy(rdb_sb[:].rearrange("p t c -> p (t c)"),
                                       rdb[:])
                        nc.vector.tensor_mul(ots[0:64, :, g, :],
                                             ot[0:64, :, 0:C], rdb_sb[0:64])
                        nc.vector.tensor_mul(ots[64:128, :, g, :],
                                             ot[64:128, :, C:2 * C],
                                             rdb_sb[64:128])
                        cc0 += nch

              # ============ PHASE 2a: O-projection (fp8-DR) ============
              with (tc.tile_pool(name="wop", bufs=1) as wop,
                    tc.tile_pool(name="sq0p", bufs=2) as sq0p,
                    tc.tile_pool(name="ps_st0", bufs=2, space="PSUM") as ps_st0):
                sum_ps0 = ps_st0.tile([1, NG * C], f32, tag="stat0")
                sq_ps0 = ps_st0.tile([1, NG * C], f32, tag="stat0")
                wo_t = load_w8(wop, wdr["ca_wo"], tag="woc")
                for m in range(MT):
                    acc = ps_mm.tile([128, 512], f32, tag="acc")
                    for kp in range(KP):
                        nc.tensor.matmul(
                            acc[:],
                            wo_t[:, 2 * kp:2 * kp + 2, m * 128:(m + 1) * 128],
                            ots[:, 2 * kp:2 * kp + 2, :, :].rearrange(
                                "p a g c -> p a (g c)"),
                            start=(kp == 0), stop=(kp == KP - 1),
                            perf_mode=DR)
                    nc.scalar.activation(x2t[:, m, :], acc[:], Act.Identity,
                                         bias=bo_t[:, m:m + 1])
                    nc.vector.tensor_add(
                        x2t[:, m, :].rearrange("p (g c) -> p g c", c=C),
                        x2t[:, m, :].rearrange("p (g c) -> p g c", c=C),
                        x1t[:, m, None, :].to_broadcast((128, NG, C)))
                    sq0 = sq0p.tile([128, NG * C], f32r, tag="sq0")
                    nc.vector.tensor_mul(sq0[:], x2t[:, m, :], x2t[:, m, :])
                    nc.tensor.matmul(sum_ps0[:], ones_r[:], x2t[:, m, :],
                                     start=(m == 0), stop=(m == MT - 1))
                    nc.tensor.matmul(sq_ps0[:], ones_r[:], sq0[:],
                                     start=(m == 0), stop=(m == MT - 1))
                nc.vector.tensor_copy(sum2_sb[:], sum_ps0[:])
                nc.vector.tensor_copy(sq2_sb[:], sq_ps0[:])

            # ============ PHASE 2: FFN (bf16), output ============
            with (
                tc.tile_pool(name="p2", bufs=1) as p2,
                tc.tile_pool(name="wstr", bufs=3) as wstr,
                tc.tile_pool(name="wstr2", bufs=2) as wstr2,
                tc.tile_pool(name="sq", bufs=2) as sqp,
                tc.tile_pool(name="ps_stat", bufs=4, space="PSUM") as ps_stat,
                tc.tile_pool(name="ps_bc", bufs=2, space="PSUM") as ps_bc,
            ):
                R = NG * C  # 512 rows
                # LN3 stats were accumulated during O-proj (sum2_sb/sq2_sb)
                mean = p2.tile([1, R], f32r, tag="mean")
                with nc.allow_low_precision(reason="f32r mean/rstd for K=1 bcast matmul"):
                    nc.scalar.mul(mean[:], sum2_sb[:], 1.0 / H)
                var = p2.tile([1, R], f32, tag="var")
                nc.scalar.mul(var[:], sq2_sb[:], 1.0 / H)
                m2 = p2.tile([1, R], f32, tag="m2")
                nc.vector.tensor_mul(m2[:], mean[:], mean[:])
                nc.vector.tensor_sub(var[:], var[:], m2[:])
                rstd = p2.tile([1, R], f32r, tag="rstd")
                with nc.allow_low_precision(reason="f32r mean/rstd for K=1 bcast matmul"):
                    nc.scalar.activation(rstd[:], var[:], Act.Sqrt, bias=eps_t[0:1])
                    nc.vector.reciprocal(rstd[:], rstd[:])
                mean_b = ps_bc.tile([128, R], f32, tag="bc")
                rstd_b = ps_bc.tile([128, R], f32, tag="bc")
                nc.tensor.matmul(mean_b[:], onesrow_r[:], mean[:],
                                 start=True, stop=True)
                nc.tensor.matmul(rstd_b[:], onesrow_r[:], rstd[:],
                                 start=True, stop=True)

                y3t = p2.tile([128, KC, R], bf16, tag="y3t")
                for m in range(MT):
                    nc.vector.tensor_sub(y3t[:, m, :], x2t[:, m, :], mean_b[:])
                    nc.vector.tensor_mul(y3t[:, m, :], y3t[:, m, :], rstd_b[:])
                    nc.vector.tensor_scalar(
                        y3t[:, m, :], y3t[:, m, :],
                        scalar1=g3_t[:, m:m + 1], scalar2=b3_t[:, m:m + 1],
                        op0=mybir.AluOpType.mult, op1=mybir.AluOpType.add)

                # GEMM1: h1T [128, FM, R] bf16
                h1t = p2.tile([128, FM, R], bf16, tag="h1t")
                w1_re = w1_d.ap().rearrange("(k p) f -> p k f", p=128)
                for fm in range(FM):
                    w1c = wstr.tile([128, KC, 128], bf16, tag="w1c")
                    for k4 in range(0, KC, 4):
                        nc.sync.dma_start(w1c[:, k4:k4 + 4, :],
                                          w1_re[:, k4:k4 + 4, fm * 128:(fm + 1) * 128])
                    acc = ps_mm.tile([128, 512], f32, tag="acc")
                    for k in range(KC):
                        nc.tensor.matmul(acc[:], w1c[:, k, :], y3t[:, k, :],
                                         start=(k == 0), stop=(k == KC - 1))
                    nc.scalar.activation(h1t[:, fm, :], acc[:], Act.Relu,
                                         bias=b1_t[:, fm:fm + 1])

                # GEMM2: x3T = W2^T-chunks @ h1T + x2T + b2
                sum3 = ps_stat.tile([1, R], f32, tag="stat")
                sq3 = ps_stat.tile([1, R], f32, tag="stat")
                a_ps = ps_stat.tile([1, R], f32, tag="stat")
                x3t = p2.tile([128, MT, R], f32r, tag="x3t")
                w2_re = w2_d.ap().rearrange("(k p) f -> p k f", p=128)
                w2cs = {}

                def load_w2c(m):
                    w2c = wstr2.tile([128, FM, 128], bf16, tag="w2c")
                    for f8_ in range(0, FM, 8):
                        nc.gpsimd.dma_start(
                            w2c[:, f8_:f8_ + 8, :],
                            w2_re[:, f8_:f8_ + 8, m * 128:(m + 1) * 128])
                    w2cs[m] = w2c
                load_w2c(0)
                for m in range(MT):
                    w2c = w2cs.pop(m)
                    if m + 1 < MT:
                        load_w2c(m + 1)
                    acc = ps_mm.tile([128, 512], f32, tag="acc")
                    for fk in range(FM):
                        nc.tensor.matmul(acc[:], w2c[:, fk, :], h1t[:, fk, :],
                                         start=(fk == 0), stop=(fk == FM - 1))
                    nc.scalar.activation(x3t[:, m, :], acc[:], Act.Identity,
                                         bias=b2_t[:, m:m + 1])
                    nc.vector.tensor_add(x3t[:, m, :], x3t[:, m, :],
                                         x2t[:, m, :])
                    sq = sqp.tile([128, R], f32r, tag="sq")
                    nc.vector.tensor_mul(sq[:], x3t[:, m, :], x3t[:, m, :])
                    nc.tensor.matmul(sum3[:], ones_r[:], x3t[:, m, :],
                                     start=(m == 0), stop=(m == MT - 1))
                    nc.tensor.matmul(sq3[:], ones_r[:], sq[:],
                                     start=(m == 0), stop=(m == MT - 1))
                    nc.tensor.matmul(a_ps[:], wg_t[:, m:m + 1], x3t[:, m, :],
                                     start=(m == 0), stop=(m == MT - 1))

                # final LN + linear folded: logits = rstd*(A - mean*S1) + S2
                mean3 = p2.tile([1, R], f32, tag="mean3")
                nc.scalar.mul(mean3[:], sum3[:], 1.0 / H)
                var3 = p2.tile([1, R], f32, tag="var3")
                nc.scalar.mul(var3[:], sq3[:], 1.0 / H)
                m23 = p2.tile([1, R], f32, tag="m23")
                nc.vector.tensor_mul(m23[:], mean3[:], mean3[:])
                nc.vector.tensor_sub(var3[:], var3[:], m23[:])
                rstd3 = p2.tile([1, R], f32, tag="rstd3")
                nc.scalar.activation(rstd3[:], var3[:], Act.Sqrt, bias=eps_t[0:1])
                nc.vector.reciprocal(rstd3[:], rstd3[:])
                logits = p2.tile([1, R], f32, tag="logits")
                nc.scalar.mul(logits[:], mean3[:], -S1)
                nc.vector.tensor_add(logits[:], logits[:], a_ps[:])
                nc.vector.tensor_mul(logits[:], logits[:], rstd3[:])
                nc.scalar.add(logits[:], logits[:], S2)
                nc.sync.dma_start(out_d.ap()[None, :], logits[:])

    nc.compile()
    return nc


def _ceil16(n):
    return -(-int(n) // 16) * 16


def _prep(inputs):
    nf = np.ascontiguousarray(np.asarray(inputs["node_features"], np.float32))
    batch = np.asarray(inputs["batch"]).astype(np.int64)
    counts = np.bincount(batch, minlength=B)
    offsets = np.concatenate([[0], np.cumsum(counts)[:-1]])

    # slot packing: rank graphs by count desc; slot j holds ranks [8j, 8j+8)
    order = np.argsort(-counts, kind="stable")
    L = [_ceil16(max(int(counts[order[j * NCORES]]), 16)) for j in range(NG)]
    offs = np.concatenate([[0], np.cumsum(L)[:-1]]).astype(int).tolist()
    nchs = [-(-l // 128) for l in L]
    TOT_ALLOC = _ceil16(offs[-1] + nchs[-1] * 128)
    meta = (offs, nchs, TOT_ALLOC)

    xts, mbs = [], []
    NCHSUM = sum(nchs)
    for c in range(NCORES):
        xt = np.zeros((H, TOT_ALLOC), np.float32)
        mbm = np.full((NCHSUM, 128), NEG, np.float32)
        cc0 = 0
        for j in range(NG):
            g = int(order[j * NCORES + c])
            n = int(counts[g])
            o = int(offsets[g])
            xt[:, offs[j]:offs[j] + n] = nf[o:o + n].T
            # empty graph guard: unmask one zero-feature key
            nv = max(n, 1)
            for ch in range(nchs[j]):
                lo = ch * 128
                mbm[cc0 + ch, :max(0, min(nv - lo, 128))] = 0.0
            cc0 += nchs[j]
        xts.append(np.clip(xt, -240.0, 240.0).astype(F8))
        mbs.append(np.ascontiguousarray(mbm.T.reshape(-1)))

    out_w = np.asarray(inputs["out_w"], np.float32)[:, 0]
    og = np.asarray(inputs["out_ln_g"], np.float32)
    ob = np.asarray(inputs["out_ln_b"], np.float32)
    wg = (out_w * og).astype(np.float32)
    S1 = float(wg.sum())
    S2 = float((out_w * ob).sum() + np.asarray(inputs["out_b"], np.float32)[0])

    def perm128(v):
        return np.asarray(v, np.float32).reshape(-1, 128).T

    pbias = np.zeros((128, 80), np.float32)
    pbias[:, 0:8] = perm128(inputs["ca_bk"])
    pbias[:, 8:16] = perm128(inputs["ca_bo"])
    pbias[:, 16:24] = perm128(inputs["ln3_g"])
    pbias[:, 24:32] = perm128(inputs["ln3_b"])
    pbias[:, 32:40] = perm128(inputs["ff_b2"])
    pbias[:, 40:48] = perm128(wg)
    pbias[:, 48:80] = perm128(inputs["ff_b1"])
    sel2 = np.zeros((2, 128), np.float32)
    sel2[0, 0:64] = 1.0
    sel2[1, 64:128] = 1.0
    common = {"cq": np.ascontiguousarray(np.asarray(inputs["class_queries"], np.float32)),
              "wg": wg, "sel2": sel2, "pbias": pbias}
    for nm in ("sa_wq", "sa_wk", "sa_wv", "sa_wo", "ca_wq", "ca_wk", "ca_wv",
               "ca_wo"):
        common[nm] = np.clip(np.asarray(inputs[nm], np.float32),
                             -240.0, 240.0).astype(F8)
    for nm in ("ff_w1", "ff_w2"):
        common[nm] = np.asarray(inputs[nm], np.float32).astype(BF16)
    for nm in ("sa_bq", "sa_bk", "sa_bv", "sa_bo", "ca_bq", "ca_bk",
               "ca_bv", "ca_bo", "ln1_g", "ln1_b", "ln2_g", "ln2_b",
               "ln3_g", "ln3_b", "ff_b1", "ff_b2"):
        common[nm] = np.ascontiguousarray(np.asarray(inputs[nm], np.float32))
    common["bias10"] = np.stack(
        [np.asarray(inputs[nm], np.float32) for nm in
         ("sa_bq", "sa_bk", "sa_bv", "sa_bo", "ca_bq",
          "ln1_g", "ln1_b", "ln2_g", "ln2_b", "ca_bv")]).astype(BF16)

    in_maps = []
    for c in range(NCORES):
        m = dict(common)
        m["xt"] = xts[c]
        m["mb"] = mbs[c]
        in_maps.append(m)
    return meta, S1, S2, in_maps, order


def _run(inputs, trace=False):
    from concourse.bass_utils import run_bass_kernel_spmd
    meta, S1, S2, in_maps, order = _prep(inputs)
    nc = build_nc(meta, S1, S2)
    try:
        r = run_bass_kernel_spmd(nc, in_maps, core_ids=list(range(NCORES)),
                                 trace=trace)
    except Exception:
        # transient device wedge (NRT_EXEC_UNIT_UNRECOVERABLE) clears on retry
        r = run_bass_kernel_spmd(nc, in_maps, core_ids=list(range(NCORES)),
                                 trace=trace)
    out = np.zeros((B, C), np.float32)
    for c in range(NCORES):
        rc = r.results[c]["out"].reshape(NG, C)
        for j in range(NG):
            out[int(order[j * NCORES + c])] = rc[j]
    return out.astype(np.float32), r


def kernel(**inputs):
    return _run(inputs, trace=False)[0]
